# revision 19
# baseline (speedup 1.0000x reference)
"""Causal self-attention Trainium2 kernel (B=4, T=2048, D=1024, H=16).

Sharding: 8 cores = 4 batches x 2 head-groups (8 heads each). Each core
computes its batch's qkv projection restricted to its 8 heads, causal
attention for those heads, and a partial out-projection over its 512 ctx
channels. Host sums the two partials per batch and adds b_out.

Per-core layout choices (all matmuls bf16 with fp32 PSUM accumulation):
  - xT [C, T]: channels on partitions (contraction dim for projections).
    Split into a [C, 512] head tile and [C, 1536] tail tile so the first
    projections only wait on ~2MB of DMA.
  - qkT: per head-pair p, a q-tile [128, T] (head A rows 0:64, head B rows
    64:128) and a k-tile [128, T]. Produced directly transposed by making
    W the stationary operand. The 1/sqrt(dk) scale is folded into Wq/bq.
  - scoresT[s, t] blocks [128, 512]: lhsT=kT (K=64 rows), rhs=qT. Heads A/B
    are row-packed (tile_position rows 0:64 / 64:128) and run concurrently.
    Diagonal blocks only compute the causally needed t-range.
  - causal mask: diagonal 128x128 squares get an extra K=128 identity
    matmul accumulating a {0, -30000} triangular pattern; exp() gives 0.
  - softmax: no max-subtraction (scores are within +-10 by construction),
    exp on ScalarE PSUM->SBUF bf16. ScalarE runs ONLY exp: the qk bias-add
    and out-proj PSUM drain live on DVE so the exp stream is never stalled
    behind table-sharing IDENTITY ops.
  - ctx: v stored naturally [s, d] with a ones column appended per head
    (v_ext [128, 8*65]); lhsT=v_ext (M=65) so PSUM row 64 accumulates the
    softmax denominator. Normalize = reciprocal_approx_fast + gpsimd
    partition_broadcast + DVE mul into the bf16 ctxT copy.
  - out projection: ctxT pair-tiles [128, T] are the stationary operand
    against W_outT; b_out is added on the host (once per batch).

Scheduling: attention blocks are exp-gated (ScalarE ~985ns/block vs PE
~740ns/block), so projection / out-projection units are spread as
"fillers" BETWEEN attention j-blocks (Bresenham) to keep the PE busy
while exp catches up. Filler map balances PE vs ScalarE per iteration:
i=0: qk/v(1); i=1: qk/v(2)+out(0); i=2: qk/v(3); i=3: out(1)+out(2);
tail: out(3). PSUM: scores 2x2 banks, ctx 2, proj/out 2.
"""

import math

import numpy as np
import ml_dtypes

B, T, C = 4, 2048, 1024
H, DK = 16, 64
NCORES = 8
TS = 128  # s-tile (partition granularity)
TSL = 512  # t free-dim tile (one PSUM bank of fp32)
MASK_VAL = -30000.0
BF16 = ml_dtypes.bfloat16


def build_program(C_sz=C, T_sz=T, n_pairs=4, num_devices=1):
    import concourse.mybir as mybir
    from concourse import bacc
    from concourse.tile import TileContext

    dt = mybir.dt
    f32 = dt.float32
    bf16 = dt.bfloat16
    AF = mybir.ActivationFunctionType

    n_ct = C_sz // 128  # contraction tiles for projections
    n_qk = 2 * n_pairs  # qk o-tiles (128 channels each)
    VW = n_pairs * 2 * DK  # v channels (natural order)
    n_tt = T_sz // TS
    n_it = T_sz // TSL
    JPI = TSL // TS  # s-tiles per i-tile (4)
    OW = min(TSL, C_sz)  # output column tile width
    n_oh = C_sz // OW  # output column halves
    VEW = n_pairs * 2 * (DK + 1)  # v_ext width (65 per head)

    nc = bacc.Bacc(
        "TRN2",
        target_bir_lowering=False,
        debug=False,
        num_devices=num_devices,
    )

    # All large inputs are pre-arranged on the host so each SBUF tile is one
    # contiguous [128, W] DMA (8KB+ per-partition lines; strided gathers from
    # DRAM are descriptor-bound and ~3x slower).
    XBW = T_sz - TSL
    xTa_d = nc.dram_tensor("xTa", [128, n_ct * TSL], bf16, kind="ExternalInput").ap()
    xTb_d = nc.dram_tensor("xTb", [128, n_ct * XBW], bf16, kind="ExternalInput").ap()
    wqk_d = nc.dram_tensor(
        "wqkT", [128, n_ct * n_qk * 128], bf16, kind="ExternalInput"
    ).ap()
    wv_d = nc.dram_tensor("wvT", [128, n_ct * VW], bf16, kind="ExternalInput").ap()
    bqk_d = nc.dram_tensor("bqk", [128, n_qk], f32, kind="ExternalInput").ap()
    bv_d = nc.dram_tensor("bv", [1, VW], bf16, kind="ExternalInput").ap()
    wo_d = nc.dram_tensor(
        "woT", [128, n_pairs * C_sz], bf16, kind="ExternalInput"
    ).ap()
    mask_d = nc.dram_tensor("masksq", [128, 2 * TS], bf16, kind="ExternalInput").ap()
    out_d = nc.dram_tensor("out", [T_sz, C_sz], f32, kind="ExternalOutput").ap()

    with TileContext(nc) as tc:
        with (
            tc.tile_pool(name="const", bufs=1) as const_pool,
            tc.tile_pool(name="big", bufs=1) as big_pool,
            tc.tile_pool(name="attn", bufs=10) as attn_pool,
            tc.tile_pool(name="rinv", bufs=6) as rinv_pool,
            tc.tile_pool(name="rbc", bufs=6) as rbc_pool,
            tc.tile_pool(name="outsb", bufs=6) as outsb_pool,
            tc.tile_pool(name="sc", bufs=2, space="PSUM") as sc_ps,
            tc.tile_pool(name="ctx", bufs=2, space="PSUM") as ctx_ps,
            tc.tile_pool(name="mm", bufs=2, space="PSUM") as mm_ps,
        ):
            # ---- input loads: one contiguous DMA per tensor, ordered so
            # the first matmuls start ASAP. v_proj(tt<4) needs wv +
            # xT[:, 0:512]; qk_proj(*, 0) additionally needs wqk. The tri
            # mask is needed by the (all-diagonal) i=0 attention blocks, so
            # it comes before the xT tail.
            def load_flat(name, dram_ap, cols):
                t = big_pool.tile([128, cols], bf16, tag=name, name=name)
                nc.sync.dma_start(t[:], dram_ap)
                return t

            wv_all = load_flat("wv", wv_d, n_ct * VW)
            xTa_all = load_flat("xTa", xTa_d, n_ct * TSL)
            bv_sb = const_pool.tile([1, VW], bf16, tag="bv", name="bv")
            nc.sync.dma_start(bv_sb[:], bv_d)
            bv_bc = const_pool.tile([128, VW], bf16, tag="bv_bc", name="bv_bc")
            nc.gpsimd.partition_broadcast(bv_bc[:], bv_sb[:])
            bqk_sb = const_pool.tile([128, n_qk], f32, tag="bqk", name="bqk")
            nc.sync.dma_start(bqk_sb[:], bqk_d)
            tri_sb = const_pool.tile([128, 2 * TS], bf16, tag="tri", name="tri")
            nc.sync.dma_start(tri_sb[:], mask_d)
            wqk_all = load_flat("wqk", wqk_d, n_ct * n_qk * 128)
            xTb_all = load_flat("xTb", xTb_d, n_ct * XBW)
            wo_all = load_flat("wo", wo_d, n_pairs * C_sz)

            def wv_sl(ci):
                return wv_all[:, ci * VW : (ci + 1) * VW]

            def wqk_sl(ci, ot):
                b = ci * n_qk * 128 + ot * 128
                return wqk_all[:, b : b + 128]

            def wo_sl(p, oh):
                b = p * C_sz + oh * OW
                return wo_all[:, b : b + OW]

            qkT_sb = [
                big_pool.tile([128, T_sz], bf16, tag=f"qkT{ot}", name=f"qkT{ot}")
                for ot in range(n_qk)
            ]
            vext_sb = [
                big_pool.tile([128, VEW], bf16, tag=f"vext{tt}", name=f"vext{tt}")
                for tt in range(n_tt)
            ]
            ctxT_sb = [
                big_pool.tile([128, T_sz], bf16, tag=f"ctxT{p}", name=f"ctxT{p}")
                for p in range(n_pairs)
            ]

            def x_cols(c0, c1):
                """AP for xT columns [c0:c1) of contraction tile ci."""
                if c1 <= TSL:
                    return lambda ci: xTa_all[:, ci * TSL + c0 : ci * TSL + c1]
                return lambda ci: xTb_all[
                    :, ci * XBW + c0 - TSL : ci * XBW + c1 - TSL
                ]

            def qk_proj(ot, i):
                xs = x_cols(i * TSL, (i + 1) * TSL)
                ps = mm_ps.tile([128, TSL], f32, tag="mm", name="mm")
                for ci in range(n_ct):
                    nc.tensor.matmul(
                        ps[:],
                        lhsT=wqk_sl(ci, ot),
                        rhs=xs(ci),
                        start=(ci == 0),
                        stop=(ci == n_ct - 1),
                    )
                nc.vector.tensor_scalar_add(
                    qkT_sb[ot][:, i * TSL : (i + 1) * TSL],
                    ps[:],
                    bqk_sb[:, ot : ot + 1],
                )

            def v_proj(tt):
                xs = x_cols(tt * TS, (tt + 1) * TS)
                ps = mm_ps.tile([128, VW], f32, tag="mm", name="mm")
                for ci in range(n_ct):
                    nc.tensor.matmul(
                        ps[:],
                        lhsT=xs(ci),
                        rhs=wv_sl(ci),
                        start=(ci == 0),
                        stop=(ci == n_ct - 1),
                    )
                vx = vext_sb[tt]
                vx3 = vx[:].rearrange("p (h e) -> p h e", e=DK + 1)
                nc.gpsimd.memset(vx3[:, :, DK : DK + 1], 1.0)
                nc.vector.scalar_tensor_tensor(
                    vx3[:, :, 0:DK],
                    ps[:].rearrange("p (h e) -> p h e", e=DK),
                    1.0,
                    bv_bc[:].rearrange("p (h e) -> p h e", e=DK),
                    op0=mybir.AluOpType.mult,
                    op1=mybir.AluOpType.add,
                )

            def out_proj(tt, oh):
                ps = mm_ps.tile([128, OW], f32, tag="mm", name="mm")
                for p in range(n_pairs):
                    nc.tensor.matmul(
                        ps[:],
                        lhsT=ctxT_sb[p][:, tt * TS : (tt + 1) * TS],
                        rhs=wo_sl(p, oh),
                        start=(p == 0),
                        stop=(p == n_pairs - 1),
                    )
                out_drain(tt, oh, ps[:])

            def out_drain(tt, oh, ps_ap):
                # drain on ScalarE (identity shares the exp table, so no
                # table reload): keeps DVE free for the latency-critical
                # diag tri-mask and normalize chains.
                ob = outsb_pool.tile([128, OW], f32, tag="outsb", name="outsb")
                nc.scalar.copy(ob[:], ps_ap)
                nc.sync.dma_start(
                    out_d[tt * TS : (tt + 1) * TS, oh * OW : (oh + 1) * OW],
                    ob[:],
                )

            def attn_pair(p, i, tick):
                qt, kt = qkT_sb[2 * p], qkT_sb[2 * p + 1]
                nj = JPI * (i + 1)
                ctxA = ctx_ps.tile([DK + 1, TSL], f32, tag="ctx", name="ctx")
                ctxB = ctx_ps.tile([DK + 1, TSL], f32, tag="ctx", name="ctx")

                def block_t0(j):
                    return (j - JPI * i) * TS if j >= JPI * i else 0

                def emit_ctx(j, a):
                    t0 = block_t0(j)
                    nc.tensor.matmul(
                        ctxA[:, t0:TSL],
                        lhsT=vext_sb[j][:, (2 * p) * (DK + 1) : (2 * p + 1) * (DK + 1)],
                        rhs=a[:, t0:TSL],
                        start=(j == 0),
                        stop=(j == nj - 1),
                    )
                    nc.tensor.matmul(
                        ctxB[:, t0:TSL],
                        lhsT=vext_sb[j][
                            :, (2 * p + 1) * (DK + 1) : (2 * p + 2) * (DK + 1)
                        ],
                        rhs=a[:, TSL + t0 : 2 * TSL],
                        start=(j == 0),
                        stop=(j == nj - 1),
                    )
                    tick()

                # One-block software pipeline: ctx(j-1) is emitted after
                # scores(j), so the in-order PE queue never heads-of-line
                # blocks on exp(j) — by the time ctx(j-1) issues, its exp
                # finished during scores(j).
                a_prev = None
                for j in range(nj):
                    diag = j >= JPI * i
                    t0 = block_t0(j)
                    ps = sc_ps.tile([128, 2 * TSL], f32, tag="sc", name="sc")
                    nc.tensor.matmul(
                        ps[:, t0:TSL],
                        lhsT=kt[0:64, j * TS : (j + 1) * TS],
                        rhs=qt[0:64, i * TSL + t0 : (i + 1) * TSL],
                        start=True,
                        stop=True,
                        skip_group_check=True,
                    )
                    nc.tensor.matmul(
                        ps[:, TSL + t0 : 2 * TSL],
                        lhsT=kt[64:128, j * TS : (j + 1) * TS],
                        rhs=qt[64:128, i * TSL + t0 : (i + 1) * TSL],
                        start=True,
                        stop=True,
                        skip_group_check=True,
                    )
                    a = attn_pool.tile([128, 2 * TSL], bf16, tag="attn", name="attn")
                    a3 = a[:].rearrange("p (c w) -> p c w", c=2)
                    ps3 = ps[:].rearrange("p (c w) -> p c w", c=2)
                    nc.scalar.activation(a3[:, :, t0:TSL], ps3[:, :, t0:TSL], AF.Exp)
                    if diag:
                        # zero the causally-dead triangle of the diagonal
                        # square (cheaper on DVE than the former identity
                        # matmul accumulating -30000 into PSUM; raw scores
                        # are bounded so exp can't overflow bf16).
                        nc.vector.tensor_tensor(
                            a3[:, :, t0 : t0 + TS],
                            a3[:, :, t0 : t0 + TS],
                            tri_sb[:].rearrange("p (c w) -> p c w", c=2),
                            op=mybir.AluOpType.mult,
                        )
                    if a_prev is not None:
                        emit_ctx(j - 1, a_prev)
                    a_prev = a
                emit_ctx(nj - 1, a_prev)
                isl = slice(i * TSL, (i + 1) * TSL)
                # A/B chains interleaved so the gpsimd broadcast of head A
                # overlaps the DVE copy/recip of head B (shorter critical
                # path into the out-projection that consumes ctxT).
                # custom-DVE ops misread PSUM on hw: bounce rowsum via SBUF.
                rcps = []
                for cps in (ctxA, ctxB):
                    rs = rinv_pool.tile([1, TSL], f32, tag="rsum", name="rsum")
                    nc.vector.tensor_copy(rs[:], cps[DK : DK + 1, :])
                    r = rinv_pool.tile([1, TSL], f32, tag="rinv", name="rinv")
                    nc.vector.reciprocal_approx_fast(r[:], rs[:])
                    rcps.append(r)
                for cps, rows, r in (
                    (ctxA, slice(0, 64), rcps[0]),
                    (ctxB, slice(64, 128), rcps[1]),
                ):
                    rbc = rbc_pool.tile([DK, TSL], f32, tag="rbc", name="rbc")
                    nc.gpsimd.partition_broadcast(rbc[:], r[:])
                    nc.vector.tensor_mul(ctxT_sb[p][rows, isl], cps[0:DK, :], rbc[:])

            # ---- software-pipelined emission ----
            for tt in range(0, JPI):
                v_proj(tt)
            for ot in range(n_qk):
                qk_proj(ot, 0)

            def filler_units(i):
                F = []
                if i + 1 < n_it:
                    qk = [lambda ot=ot: qk_proj(ot, i + 1) for ot in range(n_qk)]
                    vv = [
                        lambda tt=tt: v_proj(tt)
                        for tt in range(JPI * (i + 1), JPI * (i + 2))
                    ]
                    for a in range(JPI):
                        F.append(qk[2 * a])
                        F.append(qk[2 * a + 1])
                        F.append(vv[a])
                # out-proj batches are assigned to exp-heavy iterations:
                # out(0) during i=1, out(1)+out(2) during i=3, out(3) at end.
                outs = {1: [0], 3: [1, 2]}.get(i, [])
                for io in outs:
                    for tt in range(JPI * io, JPI * (io + 1)):
                        for oh in range(n_oh):
                            F.append(lambda tt=tt, oh=oh: out_proj(tt, oh))
                return F

            for i in range(n_it):
                F = filler_units(i)
                NB = n_pairs * JPI * (i + 1)
                nb = 0
                nf = 0

                def tick():
                    nonlocal nb, nf
                    nb += 1
                    want = len(F) * nb // NB
                    while nf < want:
                        F[nf]()
                        nf += 1

                for p in range(n_pairs):
                    # pre-pop one filler at the pair boundary: the next
                    # pair's first ctx matmul waits on the previous pair's
                    # normalize releasing the ctx PSUM tiles.
                    if nf < len(F):
                        F[nf]()
                        nf += 1
                    attn_pair(p, i, tick)
                while nf < len(F):
                    F[nf]()
                    nf += 1

            # ---- tail: out-proj of the last i-block. The final pair's
            # normalize chain (~4us on DVE/gpsimd) gates only the p=3
            # matmuls, so emit the p=0..2 accumulation chains first — they
            # execute during the normalize, keeping the PE busy and out of
            # the low p-state. 6 of 8 units get persistent PSUM (2 sc tiles
            # hold two [128,OW] halves each + 2 mm tiles); the last 2 run
            # as ordinary units.
            t0u = JPI * (n_it - 1)
            sc1 = sc_ps.tile([128, 2 * TSL], f32, tag="sc", name="sc")
            sc2 = sc_ps.tile([128, 2 * TSL], f32, tag="sc", name="sc")
            splits = [
                (t0u + 0, 0, sc1, 0),
                (t0u + 1, 0, sc1, TSL),
                (t0u + 2, 0, sc2, 0),
                (t0u + 3, 0, sc2, TSL),
                (t0u + 0, 1, mm_ps.tile([128, OW], f32, tag="mm", name="mm"), 0),
                (t0u + 1, 1, mm_ps.tile([128, OW], f32, tag="mm", name="mm"), 0),
            ]
            for tt, oh, ps, off in splits:
                for p in range(3):
                    nc.tensor.matmul(
                        ps[:, off : off + OW],
                        lhsT=ctxT_sb[p][:, tt * TS : (tt + 1) * TS],
                        rhs=wo_sl(p, oh),
                        start=(p == 0),
                        stop=False,
                        skip_group_check=True,
                    )
            for tt, oh, ps, off in splits:
                nc.tensor.matmul(
                    ps[:, off : off + OW],
                    lhsT=ctxT_sb[3][:, tt * TS : (tt + 1) * TS],
                    rhs=wo_sl(3, oh),
                    start=False,
                    stop=True,
                    skip_group_check=True,
                )
                out_drain(tt, oh, ps[:, off : off + OW])
            for tt in (t0u + 2, t0u + 3):
                out_proj(tt, 1)

    nc.compile()
    return nc


def make_tri_keep(ts=TS):
    """[128, 2*ts] duplicated keep-mask: cell (s, t) = 0 iff s > t else 1."""
    s = np.arange(128)[:, None]
    t = np.arange(ts)[None, :]
    tri = np.where(s > t, 0.0, 1.0).astype(np.float32)
    return np.concatenate([tri, tri], axis=1)


def group_rows(a, cols_slice=None):
    """[(G*128), W] -> [128, G*W]: row p holds the concat over groups g of
    a[g*128+p, :], so each SBUF tile partition is one contiguous DMA line."""
    g = a.shape[0] // 128
    if cols_slice is not None:
        a = a[:, cols_slice]
    return np.ascontiguousarray(
        a.reshape(g, 128, a.shape[1]).transpose(1, 0, 2).reshape(128, -1)
    )


def make_core_inputs(x_b, W_qkv, b_qkv, W_out, heads, C_sz=C, T_sz=T):
    """Build the per-core input map (numpy, host-side)."""
    n_pairs = len(heads) // 2
    n_qk = 2 * n_pairs
    VW = len(heads) * DK
    xT = np.ascontiguousarray(x_b.T).astype(BF16)
    wqk = np.empty((C_sz, n_qk * 128), np.float32)
    bqk = np.empty((128, n_qk), np.float32)
    wv = np.empty((C_sz, VW), np.float32)
    bv = np.empty((1, VW), np.float32)
    wo = np.empty((n_pairs * 128, C_sz), np.float32)
    for p in range(n_pairs):
        hA, hB = heads[2 * p], heads[2 * p + 1]
        # q tile (scaled by 1/sqrt(dk)=1/8), k tile
        for half, h in ((0, hA), (1, hB)):
            r0 = h * 3 * DK
            wqk[:, 2 * p * 128 + half * 64 : 2 * p * 128 + half * 64 + 64] = (
                W_qkv[r0 : r0 + DK].T / math.sqrt(DK)
            )
            bqk[half * 64 : half * 64 + 64, 2 * p] = b_qkv[r0 : r0 + DK] / math.sqrt(DK)
            wqk[:, (2 * p + 1) * 128 + half * 64 : (2 * p + 1) * 128 + half * 64 + 64] = (
                W_qkv[r0 + DK : r0 + 2 * DK].T
            )
            bqk[half * 64 : half * 64 + 64, 2 * p + 1] = b_qkv[r0 + DK : r0 + 2 * DK]
            wo[p * 128 + half * 64 : p * 128 + half * 64 + 64, :] = W_out[
                :, h * DK : (h + 1) * DK
            ].T
    for hh, h in enumerate(heads):
        r0 = h * 3 * DK + 2 * DK
        wv[:, hh * DK : (hh + 1) * DK] = W_qkv[r0 : r0 + DK].T
        bv[0, hh * DK : (hh + 1) * DK] = b_qkv[r0 : r0 + DK]
    return {
        "xTa": group_rows(xT, np.s_[0:TSL]),
        "xTb": group_rows(xT, np.s_[TSL:T_sz]),
        "wqkT": group_rows(wqk.astype(BF16)),
        "wvT": group_rows(wv.astype(BF16)),
        "bqk": bqk.astype(np.float32),
        "bv": bv.astype(BF16),
        "woT": group_rows(wo.astype(BF16)),
        "masksq": make_tri_keep().astype(BF16),
    }


_NC_CACHE = {}


def kernel(x, W_qkv, b_qkv, W_out, b_out, _trace=False):
    x = np.asarray(x, dtype=np.float32)
    W_qkv = np.asarray(W_qkv, dtype=np.float32)
    b_qkv = np.asarray(b_qkv, dtype=np.float32)
    W_out = np.asarray(W_out, dtype=np.float32)
    b_out = np.asarray(b_out, dtype=np.float32)

    from concourse.bass_utils import run_bass_kernel_spmd

    key = ("full", C, T, 4)
    if key not in _NC_CACHE:
        _NC_CACHE[key] = build_program(C, T, n_pairs=4, num_devices=1)
    nc = _NC_CACHE[key]

    in_maps = []
    for core in range(NCORES):
        b, hg = divmod(core, 2)
        heads = list(range(hg * 8, hg * 8 + 8))
        in_maps.append(make_core_inputs(x[b], W_qkv, b_qkv, W_out, heads))

    res = run_bass_kernel_spmd(nc, in_maps, list(range(NCORES)), trace=_trace)
    kernel._last_results = res

    out = np.broadcast_to(b_out, (B, T, C)).astype(np.float32).copy()
    for core in range(NCORES):
        b = core // 2
        out[b] += res.results[core]["out"]
    return out


# revision 21
# speedup vs baseline: 1.1913x; 1.1913x over previous
"""Causal self-attention Trainium2 kernel (B=4, T=2048, D=1024, H=16).

Sharding: 8 cores = 4 batches x 2 head-groups (8 heads each). Each core
computes its batch's qkv projection restricted to its 8 heads, causal
attention for those heads, and a partial out-projection over its 512 ctx
channels. Host sums the two partials per batch and adds b_out.

Per-core layout choices (all matmuls bf16 with fp32 PSUM accumulation):
  - xT [C, T]: channels on partitions (contraction dim for projections).
    Split into a [C, 512] head tile and [C, 1536] tail tile so the first
    projections only wait on ~2MB of DMA.
  - qkT: per head-pair p, a q-tile [128, T] (head A rows 0:64, head B rows
    64:128) and a k-tile [128, T]. Produced directly transposed by making
    W the stationary operand. The 1/sqrt(dk) scale is folded into Wq/bq.
  - scoresT[s, t] blocks [128, 512]: lhsT=kT (K=64 rows), rhs=qT. Heads A/B
    are row-packed (tile_position rows 0:64 / 64:128) and run concurrently.
    Diagonal blocks only compute the causally needed t-range.
  - causal mask: diagonal 128x128 squares get an extra K=128 identity
    matmul accumulating a {0, -30000} triangular pattern; exp() gives 0.
  - softmax: no max-subtraction (scores are within +-10 by construction),
    exp on ScalarE PSUM->SBUF bf16. ScalarE runs ONLY exp: the qk bias-add
    and out-proj PSUM drain live on DVE so the exp stream is never stalled
    behind table-sharing IDENTITY ops.
  - ctx: v stored naturally [s, d] with a ones column appended per head
    (v_ext [128, 8*65]); lhsT=v_ext (M=65) so PSUM row 64 accumulates the
    softmax denominator. Normalize = reciprocal_approx_fast + gpsimd
    partition_broadcast + DVE mul into the bf16 ctxT copy.
  - out projection: ctxT pair-tiles [128, T] are the stationary operand
    against W_outT; b_out is added on the host (once per batch).

Scheduling: attention blocks are exp-gated (ScalarE ~985ns/block vs PE
~740ns/block), so projection / out-projection units are spread as
"fillers" BETWEEN attention j-blocks (Bresenham) to keep the PE busy
while exp catches up. Filler map balances PE vs ScalarE per iteration:
i=0: qk/v(1); i=1: qk/v(2)+out(0); i=2: qk/v(3); i=3: out(1)+out(2);
tail: out(3). PSUM: scores 2x2 banks, ctx 2, proj/out 2.
"""

import math

import numpy as np
import ml_dtypes

B, T, C = 4, 2048, 1024
H, DK = 16, 64
NCORES = 8
TS = 128  # s-tile (partition granularity)
TSL = 512  # t free-dim tile (one PSUM bank of fp32)
MASK_VAL = -30000.0
BF16 = ml_dtypes.bfloat16


def build_program(C_sz=C, T_sz=T, n_pairs=4, num_devices=1):
    import concourse.mybir as mybir
    from concourse import bacc
    from concourse.tile import TileContext

    dt = mybir.dt
    f32 = dt.float32
    bf16 = dt.bfloat16
    AF = mybir.ActivationFunctionType

    n_ct = C_sz // 128  # contraction tiles for projections
    n_qk = 2 * n_pairs  # qk o-tiles (128 channels each)
    VW = n_pairs * 2 * DK  # v channels (natural order)
    n_tt = T_sz // TS
    n_it = T_sz // TSL
    JPI = TSL // TS  # s-tiles per i-tile (4)
    OW = min(TSL, C_sz)  # output column tile width
    n_oh = C_sz // OW  # output column halves
    VEW = n_pairs * 2 * (DK + 1)  # v_ext width (65 per head)

    nc = bacc.Bacc(
        "TRN2",
        target_bir_lowering=False,
        debug=False,
        num_devices=num_devices,
    )

    # All large inputs are pre-arranged on the host so each SBUF tile is one
    # contiguous [128, W] DMA (8KB+ per-partition lines; strided gathers from
    # DRAM are descriptor-bound and ~3x slower).
    XBW = T_sz - TSL
    xTa_d = nc.dram_tensor("xTa", [128, n_ct * TSL], bf16, kind="ExternalInput").ap()
    xTb_d = nc.dram_tensor("xTb", [128, n_ct * XBW], bf16, kind="ExternalInput").ap()
    wqk_d = nc.dram_tensor(
        "wqkT", [128, n_ct * n_qk * 128], bf16, kind="ExternalInput"
    ).ap()
    wv_d = nc.dram_tensor("wvT", [128, n_ct * VW], bf16, kind="ExternalInput").ap()
    bqk_d = nc.dram_tensor("bqk", [128, n_qk], f32, kind="ExternalInput").ap()
    bv_d = nc.dram_tensor("bv", [1, VW], bf16, kind="ExternalInput").ap()
    wo_d = nc.dram_tensor(
        "woT", [128, n_pairs * C_sz], bf16, kind="ExternalInput"
    ).ap()
    mask_d = nc.dram_tensor("masksq", [128, 2 * TS], bf16, kind="ExternalInput").ap()
    out_d = nc.dram_tensor("out", [T_sz, C_sz], f32, kind="ExternalOutput").ap()

    with TileContext(nc) as tc:
        with (
            tc.tile_pool(name="const", bufs=1) as const_pool,
            tc.tile_pool(name="big", bufs=1) as big_pool,
            tc.tile_pool(name="attn", bufs=10) as attn_pool,
            tc.tile_pool(name="rinv", bufs=6) as rinv_pool,
            tc.tile_pool(name="rbc", bufs=6) as rbc_pool,
            tc.tile_pool(name="outsb", bufs=6) as outsb_pool,
            tc.tile_pool(name="sc", bufs=2, space="PSUM") as sc_ps,
            tc.tile_pool(name="ctx", bufs=2, space="PSUM") as ctx_ps,
            tc.tile_pool(name="mm", bufs=2, space="PSUM") as mm_ps,
        ):
            # ---- input loads: one contiguous DMA per tensor, ordered so
            # the first matmuls start ASAP. v_proj(tt<4) needs wv +
            # xT[:, 0:512]; qk_proj(*, 0) additionally needs wqk. The tri
            # mask is needed by the (all-diagonal) i=0 attention blocks, so
            # it comes before the xT tail.
            def load_flat(name, dram_ap, cols):
                t = big_pool.tile([128, cols], bf16, tag=name, name=name)
                nc.sync.dma_start(t[:], dram_ap)
                return t

            wv_all = load_flat("wv", wv_d, n_ct * VW)
            xTa_all = load_flat("xTa", xTa_d, n_ct * TSL)
            bv_sb = const_pool.tile([1, VW], bf16, tag="bv", name="bv")
            nc.sync.dma_start(bv_sb[:], bv_d)
            bv_bc = const_pool.tile([128, VW], bf16, tag="bv_bc", name="bv_bc")
            nc.gpsimd.partition_broadcast(bv_bc[:], bv_sb[:])
            bqk_sb = const_pool.tile([128, n_qk], f32, tag="bqk", name="bqk")
            nc.sync.dma_start(bqk_sb[:], bqk_d)
            tri_sb = const_pool.tile([128, 2 * TS], bf16, tag="tri", name="tri")
            nc.sync.dma_start(tri_sb[:], mask_d)
            wqk_all = load_flat("wqk", wqk_d, n_ct * n_qk * 128)
            xTb_all = load_flat("xTb", xTb_d, n_ct * XBW)
            wo_all = load_flat("wo", wo_d, n_pairs * C_sz)

            def wv_sl(ci):
                return wv_all[:, ci * VW : (ci + 1) * VW]

            def wqk_sl(ci, ot):
                b = ci * n_qk * 128 + ot * 128
                return wqk_all[:, b : b + 128]

            def wo_sl(p, oh):
                b = p * C_sz + oh * OW
                return wo_all[:, b : b + OW]

            qkT_sb = [
                big_pool.tile([128, T_sz], bf16, tag=f"qkT{ot}", name=f"qkT{ot}")
                for ot in range(n_qk)
            ]
            vext_sb = [
                big_pool.tile([128, VEW], bf16, tag=f"vext{tt}", name=f"vext{tt}")
                for tt in range(n_tt)
            ]
            ctxT_sb = [
                big_pool.tile([128, T_sz], bf16, tag=f"ctxT{p}", name=f"ctxT{p}")
                for p in range(n_pairs)
            ]

            def x_cols(c0, c1):
                """AP for xT columns [c0:c1) of contraction tile ci."""
                if c1 <= TSL:
                    return lambda ci: xTa_all[:, ci * TSL + c0 : ci * TSL + c1]
                return lambda ci: xTb_all[
                    :, ci * XBW + c0 - TSL : ci * XBW + c1 - TSL
                ]

            def qk_proj(ot, i):
                xs = x_cols(i * TSL, (i + 1) * TSL)
                ps = mm_ps.tile([128, TSL], f32, tag="mm", name="mm")
                for ci in range(n_ct):
                    nc.tensor.matmul(
                        ps[:],
                        lhsT=wqk_sl(ci, ot),
                        rhs=xs(ci),
                        start=(ci == 0),
                        stop=(ci == n_ct - 1),
                    )
                nc.vector.tensor_scalar_add(
                    qkT_sb[ot][:, i * TSL : (i + 1) * TSL],
                    ps[:],
                    bqk_sb[:, ot : ot + 1],
                )

            def v_proj(tt):
                xs = x_cols(tt * TS, (tt + 1) * TS)
                ps = mm_ps.tile([128, VW], f32, tag="mm", name="mm")
                for ci in range(n_ct):
                    nc.tensor.matmul(
                        ps[:],
                        lhsT=xs(ci),
                        rhs=wv_sl(ci),
                        start=(ci == 0),
                        stop=(ci == n_ct - 1),
                    )
                vx = vext_sb[tt]
                vx3 = vx[:].rearrange("p (h e) -> p h e", e=DK + 1)
                nc.gpsimd.memset(vx3[:, :, DK : DK + 1], 1.0)
                nc.vector.scalar_tensor_tensor(
                    vx3[:, :, 0:DK],
                    ps[:].rearrange("p (h e) -> p h e", e=DK),
                    1.0,
                    bv_bc[:].rearrange("p (h e) -> p h e", e=DK),
                    op0=mybir.AluOpType.mult,
                    op1=mybir.AluOpType.add,
                )

            def out_proj(tt, oh):
                ps = mm_ps.tile([128, OW], f32, tag="mm", name="mm")
                for p in range(n_pairs):
                    nc.tensor.matmul(
                        ps[:],
                        lhsT=ctxT_sb[p][:, tt * TS : (tt + 1) * TS],
                        rhs=wo_sl(p, oh),
                        start=(p == 0),
                        stop=(p == n_pairs - 1),
                    )
                out_drain(tt, oh, ps[:])

            def out_drain(tt, oh, ps_ap):
                ob = outsb_pool.tile([128, OW], f32, tag="outsb", name="outsb")
                nc.vector.tensor_copy(ob[:], ps_ap)
                nc.sync.dma_start(
                    out_d[tt * TS : (tt + 1) * TS, oh * OW : (oh + 1) * OW],
                    ob[:],
                )

            def attn_pair(p, i, tick):
                qt, kt = qkT_sb[2 * p], qkT_sb[2 * p + 1]
                nj = JPI * (i + 1)
                ctxA = ctx_ps.tile([DK + 1, TSL], f32, tag="ctx", name="ctx")
                ctxB = ctx_ps.tile([DK + 1, TSL], f32, tag="ctx", name="ctx")

                def block_t0(j):
                    return (j - JPI * i) * TS if j >= JPI * i else 0

                def emit_ctx(j, a):
                    t0 = block_t0(j)
                    nc.tensor.matmul(
                        ctxA[:, t0:TSL],
                        lhsT=vext_sb[j][:, (2 * p) * (DK + 1) : (2 * p + 1) * (DK + 1)],
                        rhs=a[:, t0:TSL],
                        start=(j == 0),
                        stop=(j == nj - 1),
                    )
                    nc.tensor.matmul(
                        ctxB[:, t0:TSL],
                        lhsT=vext_sb[j][
                            :, (2 * p + 1) * (DK + 1) : (2 * p + 2) * (DK + 1)
                        ],
                        rhs=a[:, TSL + t0 : 2 * TSL],
                        start=(j == 0),
                        stop=(j == nj - 1),
                    )
                    tick()

                # One-block software pipeline: ctx(j-1) is emitted after
                # scores(j), so the in-order PE queue never heads-of-line
                # blocks on exp(j) — by the time ctx(j-1) issues, its exp
                # finished during scores(j).
                a_prev = None
                for j in range(nj):
                    diag = j >= JPI * i
                    t0 = block_t0(j)
                    ps = sc_ps.tile([128, 2 * TSL], f32, tag="sc", name="sc")
                    nc.tensor.matmul(
                        ps[:, t0:TSL],
                        lhsT=kt[0:64, j * TS : (j + 1) * TS],
                        rhs=qt[0:64, i * TSL + t0 : (i + 1) * TSL],
                        start=True,
                        stop=True,
                        skip_group_check=True,
                    )
                    nc.tensor.matmul(
                        ps[:, TSL + t0 : 2 * TSL],
                        lhsT=kt[64:128, j * TS : (j + 1) * TS],
                        rhs=qt[64:128, i * TSL + t0 : (i + 1) * TSL],
                        start=True,
                        stop=True,
                        skip_group_check=True,
                    )
                    a = attn_pool.tile([128, 2 * TSL], bf16, tag="attn", name="attn")
                    a3 = a[:].rearrange("p (c w) -> p c w", c=2)
                    ps3 = ps[:].rearrange("p (c w) -> p c w", c=2)
                    nc.scalar.activation(a3[:, :, t0:TSL], ps3[:, :, t0:TSL], AF.Exp)
                    if diag:
                        # zero the causally-dead triangle of the diagonal
                        # square (cheaper on DVE than the former identity
                        # matmul accumulating -30000 into PSUM; raw scores
                        # are bounded so exp can't overflow bf16).
                        nc.vector.tensor_tensor(
                            a3[:, :, t0 : t0 + TS],
                            a3[:, :, t0 : t0 + TS],
                            tri_sb[:].rearrange("p (c w) -> p c w", c=2),
                            op=mybir.AluOpType.mult,
                        )
                    if a_prev is not None:
                        emit_ctx(j - 1, a_prev)
                    a_prev = a
                emit_ctx(nj - 1, a_prev)
                isl = slice(i * TSL, (i + 1) * TSL)
                # A/B chains interleaved so the gpsimd broadcast of head A
                # overlaps the DVE copy/recip of head B (shorter critical
                # path into the out-projection that consumes ctxT).
                # custom-DVE ops misread PSUM on hw: bounce rowsum via SBUF.
                rcps = []
                for cps in (ctxA, ctxB):
                    rs = rinv_pool.tile([1, TSL], f32, tag="rsum", name="rsum")
                    nc.vector.tensor_copy(rs[:], cps[DK : DK + 1, :])
                    r = rinv_pool.tile([1, TSL], f32, tag="rinv", name="rinv")
                    nc.vector.reciprocal_approx_fast(r[:], rs[:])
                    rcps.append(r)
                for cps, rows, r in (
                    (ctxA, slice(0, 64), rcps[0]),
                    (ctxB, slice(64, 128), rcps[1]),
                ):
                    rbc = rbc_pool.tile([DK, TSL], f32, tag="rbc", name="rbc")
                    nc.gpsimd.partition_broadcast(rbc[:], r[:])
                    nc.vector.tensor_mul(ctxT_sb[p][rows, isl], cps[0:DK, :], rbc[:])

            # ---- software-pipelined emission ----
            for tt in range(0, JPI):
                v_proj(tt)
            for ot in range(n_qk):
                qk_proj(ot, 0)

            def filler_units(i):
                F = []
                if i + 1 < n_it:
                    qk = [lambda ot=ot: qk_proj(ot, i + 1) for ot in range(n_qk)]
                    vv = [
                        lambda tt=tt: v_proj(tt)
                        for tt in range(JPI * (i + 1), JPI * (i + 2))
                    ]
                    for a in range(JPI):
                        F.append(qk[2 * a])
                        F.append(qk[2 * a + 1])
                        F.append(vv[a])
                # out-proj batches are assigned to exp-heavy iterations:
                # out(0) during i=1, out(1)+out(2) during i=3, out(3) at end.
                outs = {1: [0], 3: [1, 2]}.get(i, [])
                for io in outs:
                    for tt in range(JPI * io, JPI * (io + 1)):
                        for oh in range(n_oh):
                            F.append(lambda tt=tt, oh=oh: out_proj(tt, oh))
                return F

            for i in range(n_it):
                F = filler_units(i)
                NB = n_pairs * JPI * (i + 1)
                nb = 0
                nf = 0

                def tick():
                    nonlocal nb, nf
                    nb += 1
                    want = len(F) * nb // NB
                    while nf < want:
                        F[nf]()
                        nf += 1

                for p in range(n_pairs):
                    attn_pair(p, i, tick)
                while nf < len(F):
                    F[nf]()
                    nf += 1

            # ---- tail: out-proj of the last i-block. The final pair's
            # normalize chain (~4us on DVE/gpsimd) gates only the p=3
            # matmuls, so emit the p=0..2 accumulation chains first — they
            # execute during the normalize, keeping the PE busy and out of
            # the low p-state. 6 of 8 units get persistent PSUM (2 sc tiles
            # hold two [128,OW] halves each + 2 mm tiles); the last 2 run
            # as ordinary units.
            t0u = JPI * (n_it - 1)
            sc1 = sc_ps.tile([128, 2 * TSL], f32, tag="sc", name="sc")
            sc2 = sc_ps.tile([128, 2 * TSL], f32, tag="sc", name="sc")
            splits = [
                (t0u + 0, 0, sc1, 0),
                (t0u + 1, 0, sc1, TSL),
                (t0u + 2, 0, sc2, 0),
                (t0u + 3, 0, sc2, TSL),
                (t0u + 0, 1, mm_ps.tile([128, OW], f32, tag="mm", name="mm"), 0),
                (t0u + 1, 1, mm_ps.tile([128, OW], f32, tag="mm", name="mm"), 0),
            ]
            for tt, oh, ps, off in splits:
                for p in range(3):
                    nc.tensor.matmul(
                        ps[:, off : off + OW],
                        lhsT=ctxT_sb[p][:, tt * TS : (tt + 1) * TS],
                        rhs=wo_sl(p, oh),
                        start=(p == 0),
                        stop=False,
                        skip_group_check=True,
                    )
            for tt, oh, ps, off in splits:
                nc.tensor.matmul(
                    ps[:, off : off + OW],
                    lhsT=ctxT_sb[3][:, tt * TS : (tt + 1) * TS],
                    rhs=wo_sl(3, oh),
                    start=False,
                    stop=True,
                    skip_group_check=True,
                )
                out_drain(tt, oh, ps[:, off : off + OW])
            for tt in (t0u + 2, t0u + 3):
                out_proj(tt, 1)

    nc.compile()
    return nc


def make_tri_keep(ts=TS):
    """[128, 2*ts] duplicated keep-mask: cell (s, t) = 0 iff s > t else 1."""
    s = np.arange(128)[:, None]
    t = np.arange(ts)[None, :]
    tri = np.where(s > t, 0.0, 1.0).astype(np.float32)
    return np.concatenate([tri, tri], axis=1)


def group_rows(a, cols_slice=None):
    """[(G*128), W] -> [128, G*W]: row p holds the concat over groups g of
    a[g*128+p, :], so each SBUF tile partition is one contiguous DMA line."""
    g = a.shape[0] // 128
    if cols_slice is not None:
        a = a[:, cols_slice]
    return np.ascontiguousarray(
        a.reshape(g, 128, a.shape[1]).transpose(1, 0, 2).reshape(128, -1)
    )


def make_core_inputs(x_b, W_qkv, b_qkv, W_out, heads, C_sz=C, T_sz=T):
    """Build the per-core input map (numpy, host-side)."""
    n_pairs = len(heads) // 2
    n_qk = 2 * n_pairs
    VW = len(heads) * DK
    xT = np.ascontiguousarray(x_b.T).astype(BF16)
    wqk = np.empty((C_sz, n_qk * 128), np.float32)
    bqk = np.empty((128, n_qk), np.float32)
    wv = np.empty((C_sz, VW), np.float32)
    bv = np.empty((1, VW), np.float32)
    wo = np.empty((n_pairs * 128, C_sz), np.float32)
    for p in range(n_pairs):
        hA, hB = heads[2 * p], heads[2 * p + 1]
        # q tile (scaled by 1/sqrt(dk)=1/8), k tile
        for half, h in ((0, hA), (1, hB)):
            r0 = h * 3 * DK
            wqk[:, 2 * p * 128 + half * 64 : 2 * p * 128 + half * 64 + 64] = (
                W_qkv[r0 : r0 + DK].T / math.sqrt(DK)
            )
            bqk[half * 64 : half * 64 + 64, 2 * p] = b_qkv[r0 : r0 + DK] / math.sqrt(DK)
            wqk[:, (2 * p + 1) * 128 + half * 64 : (2 * p + 1) * 128 + half * 64 + 64] = (
                W_qkv[r0 + DK : r0 + 2 * DK].T
            )
            bqk[half * 64 : half * 64 + 64, 2 * p + 1] = b_qkv[r0 + DK : r0 + 2 * DK]
            wo[p * 128 + half * 64 : p * 128 + half * 64 + 64, :] = W_out[
                :, h * DK : (h + 1) * DK
            ].T
    for hh, h in enumerate(heads):
        r0 = h * 3 * DK + 2 * DK
        wv[:, hh * DK : (hh + 1) * DK] = W_qkv[r0 : r0 + DK].T
        bv[0, hh * DK : (hh + 1) * DK] = b_qkv[r0 : r0 + DK]
    return {
        "xTa": group_rows(xT, np.s_[0:TSL]),
        "xTb": group_rows(xT, np.s_[TSL:T_sz]),
        "wqkT": group_rows(wqk.astype(BF16)),
        "wvT": group_rows(wv.astype(BF16)),
        "bqk": bqk.astype(np.float32),
        "bv": bv.astype(BF16),
        "woT": group_rows(wo.astype(BF16)),
        "masksq": make_tri_keep().astype(BF16),
    }


_NC_CACHE = {}


def kernel(x, W_qkv, b_qkv, W_out, b_out, _trace=False):
    x = np.asarray(x, dtype=np.float32)
    W_qkv = np.asarray(W_qkv, dtype=np.float32)
    b_qkv = np.asarray(b_qkv, dtype=np.float32)
    W_out = np.asarray(W_out, dtype=np.float32)
    b_out = np.asarray(b_out, dtype=np.float32)

    from concourse.bass_utils import run_bass_kernel_spmd

    key = ("full", C, T, 4)
    if key not in _NC_CACHE:
        _NC_CACHE[key] = build_program(C, T, n_pairs=4, num_devices=1)
    nc = _NC_CACHE[key]

    in_maps = []
    for core in range(NCORES):
        b, hg = divmod(core, 2)
        heads = list(range(hg * 8, hg * 8 + 8))
        in_maps.append(make_core_inputs(x[b], W_qkv, b_qkv, W_out, heads))

    res = run_bass_kernel_spmd(nc, in_maps, list(range(NCORES)), trace=_trace)
    kernel._last_results = res

    out = np.broadcast_to(b_out, (B, T, C)).astype(np.float32).copy()
    for core in range(NCORES):
        b = core // 2
        out[b] += res.results[core]["out"]
    return out


# revision 22
# speedup vs baseline: 1.1944x; 1.0026x over previous
"""Causal self-attention Trainium2 kernel (B=4, T=2048, D=1024, H=16).

Sharding: 8 cores = 4 batches x 2 head-groups (8 heads each). Each core
computes its batch's qkv projection restricted to its 8 heads, causal
attention for those heads, and a partial out-projection over its 512 ctx
channels. Host sums the two partials per batch and adds b_out.

Per-core layout choices (all matmuls bf16 with fp32 PSUM accumulation):
  - xT [C, T]: channels on partitions (contraction dim for projections).
    Split into a [C, 512] head tile and [C, 1536] tail tile so the first
    projections only wait on ~2MB of DMA.
  - qkT: per head-pair p, a q-tile [128, T] (head A rows 0:64, head B rows
    64:128) and a k-tile [128, T]. Produced directly transposed by making
    W the stationary operand. The 1/sqrt(dk) scale is folded into Wq/bq.
  - scoresT[s, t] blocks [128, 512]: lhsT=kT (K=64 rows), rhs=qT. Heads A/B
    are row-packed (tile_position rows 0:64 / 64:128) and run concurrently.
    Diagonal blocks only compute the causally needed t-range.
  - causal mask: diagonal 128x128 squares get an extra K=128 identity
    matmul accumulating a {0, -30000} triangular pattern; exp() gives 0.
  - softmax: no max-subtraction (scores are within +-10 by construction),
    exp on ScalarE PSUM->SBUF bf16. ScalarE runs ONLY exp: the qk bias-add
    and out-proj PSUM drain live on DVE so the exp stream is never stalled
    behind table-sharing IDENTITY ops.
  - ctx: v stored naturally [s, d] with a ones column appended per head
    (v_ext [128, 8*65]); lhsT=v_ext (M=65) so PSUM row 64 accumulates the
    softmax denominator. Normalize = reciprocal_approx_fast + gpsimd
    partition_broadcast + DVE mul into the bf16 ctxT copy.
  - out projection: ctxT pair-tiles [128, T] are the stationary operand
    against W_outT; b_out is added on the host (once per batch).

Scheduling: attention blocks are exp-gated (ScalarE ~985ns/block vs PE
~740ns/block), so projection / out-projection units are spread as
"fillers" BETWEEN attention j-blocks (Bresenham) to keep the PE busy
while exp catches up. Filler map balances PE vs ScalarE per iteration:
i=0: qk/v(1); i=1: qk/v(2)+out(0); i=2: qk/v(3); i=3: out(1)+out(2);
tail: out(3). PSUM: scores 2x2 banks, ctx 2, proj/out 2.
"""

import math

import numpy as np
import ml_dtypes

B, T, C = 4, 2048, 1024
H, DK = 16, 64
NCORES = 8
TS = 128  # s-tile (partition granularity)
TSL = 512  # t free-dim tile (one PSUM bank of fp32)
MASK_VAL = -30000.0
BF16 = ml_dtypes.bfloat16


def build_program(C_sz=C, T_sz=T, n_pairs=4, num_devices=1):
    import concourse.mybir as mybir
    from concourse import bacc
    from concourse.tile import TileContext

    dt = mybir.dt
    f32 = dt.float32
    bf16 = dt.bfloat16
    AF = mybir.ActivationFunctionType

    n_ct = C_sz // 128  # contraction tiles for projections
    n_qk = 2 * n_pairs  # qk o-tiles (128 channels each)
    VW = n_pairs * 2 * DK  # v channels (natural order)
    n_tt = T_sz // TS
    n_it = T_sz // TSL
    JPI = TSL // TS  # s-tiles per i-tile (4)
    OW = min(TSL, C_sz)  # output column tile width
    n_oh = C_sz // OW  # output column halves
    VEW = n_pairs * 2 * (DK + 1)  # v_ext width (65 per head)

    nc = bacc.Bacc(
        "TRN2",
        target_bir_lowering=False,
        debug=False,
        num_devices=num_devices,
    )

    # All large inputs are pre-arranged on the host so each SBUF tile is one
    # contiguous [128, W] DMA (8KB+ per-partition lines; strided gathers from
    # DRAM are descriptor-bound and ~3x slower).
    XBW = T_sz - TSL
    xTa_d = nc.dram_tensor("xTa", [128, n_ct * TSL], bf16, kind="ExternalInput").ap()
    xTb_d = nc.dram_tensor("xTb", [128, n_ct * XBW], bf16, kind="ExternalInput").ap()
    wqk_d = nc.dram_tensor(
        "wqkT", [128, n_ct * n_qk * 128], bf16, kind="ExternalInput"
    ).ap()
    wv_d = nc.dram_tensor("wvT", [128, n_ct * VW], bf16, kind="ExternalInput").ap()
    bqk_d = nc.dram_tensor("bqk", [128, n_qk], f32, kind="ExternalInput").ap()
    bv_d = nc.dram_tensor("bv", [1, VW], bf16, kind="ExternalInput").ap()
    wo_d = nc.dram_tensor(
        "woT", [128, n_pairs * C_sz], bf16, kind="ExternalInput"
    ).ap()
    mask_d = nc.dram_tensor("masksq", [128, 2 * TS], bf16, kind="ExternalInput").ap()
    out_d = nc.dram_tensor("out", [T_sz, C_sz], f32, kind="ExternalOutput").ap()

    with TileContext(nc) as tc:
        with (
            tc.tile_pool(name="const", bufs=1) as const_pool,
            tc.tile_pool(name="big", bufs=1) as big_pool,
            tc.tile_pool(name="attn", bufs=10) as attn_pool,
            tc.tile_pool(name="rinv", bufs=6) as rinv_pool,
            tc.tile_pool(name="rbc", bufs=6) as rbc_pool,
            tc.tile_pool(name="outsb", bufs=6) as outsb_pool,
            tc.tile_pool(name="sc", bufs=2, space="PSUM") as sc_ps,
            tc.tile_pool(name="ctx", bufs=2, space="PSUM") as ctx_ps,
            tc.tile_pool(name="mm", bufs=2, space="PSUM") as mm_ps,
        ):
            # ---- input loads: one contiguous DMA per tensor, ordered so
            # the first matmuls start ASAP. v_proj(tt<4) needs wv +
            # xT[:, 0:512]; qk_proj(*, 0) additionally needs wqk. The tri
            # mask is needed by the (all-diagonal) i=0 attention blocks, so
            # it comes before the xT tail.
            def load_flat(name, dram_ap, cols):
                t = big_pool.tile([128, cols], bf16, tag=name, name=name)
                nc.sync.dma_start(t[:], dram_ap)
                return t

            wv_all = load_flat("wv", wv_d, n_ct * VW)
            xTa_all = load_flat("xTa", xTa_d, n_ct * TSL)
            bv_sb = const_pool.tile([1, VW], bf16, tag="bv", name="bv")
            nc.sync.dma_start(bv_sb[:], bv_d)
            bv_bc = const_pool.tile([128, VW], bf16, tag="bv_bc", name="bv_bc")
            nc.gpsimd.partition_broadcast(bv_bc[:], bv_sb[:])
            bqk_sb = const_pool.tile([128, n_qk], f32, tag="bqk", name="bqk")
            nc.sync.dma_start(bqk_sb[:], bqk_d)
            tri_sb = const_pool.tile([128, 2 * TS], bf16, tag="tri", name="tri")
            nc.sync.dma_start(tri_sb[:], mask_d)
            wqk_all = load_flat("wqk", wqk_d, n_ct * n_qk * 128)
            xTb_all = load_flat("xTb", xTb_d, n_ct * XBW)
            wo_all = load_flat("wo", wo_d, n_pairs * C_sz)

            def wv_sl(ci):
                return wv_all[:, ci * VW : (ci + 1) * VW]

            def wqk_sl(ci, ot):
                b = ci * n_qk * 128 + ot * 128
                return wqk_all[:, b : b + 128]

            def wo_sl(p, oh):
                b = p * C_sz + oh * OW
                return wo_all[:, b : b + OW]

            qkT_sb = [
                big_pool.tile([128, T_sz], bf16, tag=f"qkT{ot}", name=f"qkT{ot}")
                for ot in range(n_qk)
            ]
            vext_sb = [
                big_pool.tile([128, VEW], bf16, tag=f"vext{tt}", name=f"vext{tt}")
                for tt in range(n_tt)
            ]
            ctxT_sb = [
                big_pool.tile([128, T_sz], bf16, tag=f"ctxT{p}", name=f"ctxT{p}")
                for p in range(n_pairs)
            ]

            def x_cols(c0, c1):
                """AP for xT columns [c0:c1) of contraction tile ci."""
                if c1 <= TSL:
                    return lambda ci: xTa_all[:, ci * TSL + c0 : ci * TSL + c1]
                return lambda ci: xTb_all[
                    :, ci * XBW + c0 - TSL : ci * XBW + c1 - TSL
                ]

            def qk_proj(ot, i):
                xs = x_cols(i * TSL, (i + 1) * TSL)
                ps = mm_ps.tile([128, TSL], f32, tag="mm", name="mm")
                for ci in range(n_ct):
                    nc.tensor.matmul(
                        ps[:],
                        lhsT=wqk_sl(ci, ot),
                        rhs=xs(ci),
                        start=(ci == 0),
                        stop=(ci == n_ct - 1),
                    )
                nc.vector.tensor_scalar_add(
                    qkT_sb[ot][:, i * TSL : (i + 1) * TSL],
                    ps[:],
                    bqk_sb[:, ot : ot + 1],
                )

            def v_proj(tt):
                xs = x_cols(tt * TS, (tt + 1) * TS)
                ps = mm_ps.tile([128, VW], f32, tag="mm", name="mm")
                for ci in range(n_ct):
                    nc.tensor.matmul(
                        ps[:],
                        lhsT=xs(ci),
                        rhs=wv_sl(ci),
                        start=(ci == 0),
                        stop=(ci == n_ct - 1),
                    )
                vx = vext_sb[tt]
                vx3 = vx[:].rearrange("p (h e) -> p h e", e=DK + 1)
                nc.gpsimd.memset(vx3[:, :, DK : DK + 1], 1.0)
                nc.vector.scalar_tensor_tensor(
                    vx3[:, :, 0:DK],
                    ps[:].rearrange("p (h e) -> p h e", e=DK),
                    1.0,
                    bv_bc[:].rearrange("p (h e) -> p h e", e=DK),
                    op0=mybir.AluOpType.mult,
                    op1=mybir.AluOpType.add,
                )

            def out_proj(tt, oh):
                ps = mm_ps.tile([128, OW], f32, tag="mm", name="mm")
                for p in range(n_pairs):
                    nc.tensor.matmul(
                        ps[:],
                        lhsT=ctxT_sb[p][:, tt * TS : (tt + 1) * TS],
                        rhs=wo_sl(p, oh),
                        start=(p == 0),
                        stop=(p == n_pairs - 1),
                    )
                out_drain(tt, oh, ps[:])

            def out_drain(tt, oh, ps_ap):
                ob = outsb_pool.tile([128, OW], f32, tag="outsb", name="outsb")
                nc.vector.tensor_copy(ob[:], ps_ap)
                nc.sync.dma_start(
                    out_d[tt * TS : (tt + 1) * TS, oh * OW : (oh + 1) * OW],
                    ob[:],
                )

            def attn_pair(p, i, tick):
                qt, kt = qkT_sb[2 * p], qkT_sb[2 * p + 1]
                nj = JPI * (i + 1)
                ctxA = ctx_ps.tile([DK + 1, TSL], f32, tag="ctx", name="ctx")
                ctxB = ctx_ps.tile([DK + 1, TSL], f32, tag="ctx", name="ctx")

                def block_t0(j):
                    return (j - JPI * i) * TS if j >= JPI * i else 0

                def emit_ctx(j, a):
                    t0 = block_t0(j)
                    nc.tensor.matmul(
                        ctxA[:, t0:TSL],
                        lhsT=vext_sb[j][:, (2 * p) * (DK + 1) : (2 * p + 1) * (DK + 1)],
                        rhs=a[:, t0:TSL],
                        start=(j == 0),
                        stop=(j == nj - 1),
                    )
                    nc.tensor.matmul(
                        ctxB[:, t0:TSL],
                        lhsT=vext_sb[j][
                            :, (2 * p + 1) * (DK + 1) : (2 * p + 2) * (DK + 1)
                        ],
                        rhs=a[:, TSL + t0 : 2 * TSL],
                        start=(j == 0),
                        stop=(j == nj - 1),
                    )
                    tick()

                # One-block software pipeline: ctx(j-1) is emitted after
                # scores(j), so the in-order PE queue never heads-of-line
                # blocks on exp(j) — by the time ctx(j-1) issues, its exp
                # finished during scores(j).
                a_prev = None
                for j in range(nj):
                    diag = j >= JPI * i
                    t0 = block_t0(j)
                    ps = sc_ps.tile([128, 2 * TSL], f32, tag="sc", name="sc")
                    nc.tensor.matmul(
                        ps[:, t0:TSL],
                        lhsT=kt[0:64, j * TS : (j + 1) * TS],
                        rhs=qt[0:64, i * TSL + t0 : (i + 1) * TSL],
                        start=True,
                        stop=True,
                        skip_group_check=True,
                    )
                    nc.tensor.matmul(
                        ps[:, TSL + t0 : 2 * TSL],
                        lhsT=kt[64:128, j * TS : (j + 1) * TS],
                        rhs=qt[64:128, i * TSL + t0 : (i + 1) * TSL],
                        start=True,
                        stop=True,
                        skip_group_check=True,
                    )
                    a = attn_pool.tile([128, 2 * TSL], bf16, tag="attn", name="attn")
                    a3 = a[:].rearrange("p (c w) -> p c w", c=2)
                    ps3 = ps[:].rearrange("p (c w) -> p c w", c=2)
                    nc.scalar.activation(a3[:, :, t0:TSL], ps3[:, :, t0:TSL], AF.Exp)
                    if diag:
                        # zero the causally-dead triangle of the diagonal
                        # square (cheaper on DVE than the former identity
                        # matmul accumulating -30000 into PSUM; raw scores
                        # are bounded so exp can't overflow bf16).
                        nc.vector.tensor_tensor(
                            a3[:, :, t0 : t0 + TS],
                            a3[:, :, t0 : t0 + TS],
                            tri_sb[:].rearrange("p (c w) -> p c w", c=2),
                            op=mybir.AluOpType.mult,
                        )
                    if a_prev is not None:
                        emit_ctx(j - 1, a_prev)
                    a_prev = a
                emit_ctx(nj - 1, a_prev)
                isl = slice(i * TSL, (i + 1) * TSL)
                # A/B chains interleaved so the gpsimd broadcast of head A
                # overlaps the DVE copy/recip of head B (shorter critical
                # path into the out-projection that consumes ctxT).
                # custom-DVE ops misread PSUM on hw: bounce rowsum via SBUF.
                rcps = []
                for cps in (ctxA, ctxB):
                    rs = rinv_pool.tile([1, TSL], f32, tag="rsum", name="rsum")
                    nc.vector.tensor_copy(rs[:], cps[DK : DK + 1, :])
                    r = rinv_pool.tile([1, TSL], f32, tag="rinv", name="rinv")
                    nc.vector.reciprocal_approx_fast(r[:], rs[:])
                    rcps.append(r)
                for cps, rows, r in (
                    (ctxA, slice(0, 64), rcps[0]),
                    (ctxB, slice(64, 128), rcps[1]),
                ):
                    rbc = rbc_pool.tile([DK, TSL], f32, tag="rbc", name="rbc")
                    nc.gpsimd.partition_broadcast(rbc[:], r[:])
                    nc.vector.tensor_mul(ctxT_sb[p][rows, isl], cps[0:DK, :], rbc[:])

            # ---- software-pipelined emission ----
            for tt in range(0, JPI):
                v_proj(tt)
            for ot in range(n_qk):
                qk_proj(ot, 0)

            def filler_units(i):
                F = []
                if i + 1 < n_it:
                    qk = [lambda ot=ot: qk_proj(ot, i + 1) for ot in range(n_qk)]
                    vv = [
                        lambda tt=tt: v_proj(tt)
                        for tt in range(JPI * (i + 1), JPI * (i + 2))
                    ]
                    for a in range(JPI):
                        F.append(qk[2 * a])
                        F.append(qk[2 * a + 1])
                        F.append(vv[a])
                # out-proj batches are assigned to exp-heavy iterations:
                # out(0) during i=1, out(1)+out(2) during i=3, out(3) at end.
                outs = {1: [0], 3: [1, 2]}.get(i, [])
                for io in outs:
                    for tt in range(JPI * io, JPI * (io + 1)):
                        for oh in range(n_oh):
                            F.append(lambda tt=tt, oh=oh: out_proj(tt, oh))
                return F

            for i in range(n_it):
                F = filler_units(i)
                NB = n_pairs * JPI * (i + 1)
                nb = 0
                nf = 0

                def tick():
                    nonlocal nb, nf
                    nb += 1
                    if i == n_it - 1:
                        # back-weighted spread: ScalarE's exp backlog peaks at
                        # the end of the last iteration, so save fillers for it
                        want = len(F) * nb * nb // (NB * NB)
                    else:
                        want = len(F) * nb // NB
                    while nf < want:
                        F[nf]()
                        nf += 1

                for p in range(n_pairs):
                    attn_pair(p, i, tick)
                while nf < len(F):
                    F[nf]()
                    nf += 1

            # ---- tail: out-proj of the last i-block. The final pair's
            # normalize chain (~4us on DVE/gpsimd) gates only the p=3
            # matmuls, so emit the p=0..2 accumulation chains first — they
            # execute during the normalize, keeping the PE busy and out of
            # the low p-state. 6 of 8 units get persistent PSUM (2 sc tiles
            # hold two [128,OW] halves each + 2 mm tiles); the last 2 run
            # as ordinary units.
            t0u = JPI * (n_it - 1)
            sc1 = sc_ps.tile([128, 2 * TSL], f32, tag="sc", name="sc")
            sc2 = sc_ps.tile([128, 2 * TSL], f32, tag="sc", name="sc")
            splits = [
                (t0u + 0, 0, sc1, 0),
                (t0u + 1, 0, sc1, TSL),
                (t0u + 2, 0, sc2, 0),
                (t0u + 3, 0, sc2, TSL),
                (t0u + 0, 1, mm_ps.tile([128, OW], f32, tag="mm", name="mm"), 0),
                (t0u + 1, 1, mm_ps.tile([128, OW], f32, tag="mm", name="mm"), 0),
            ]
            for tt, oh, ps, off in splits:
                for p in range(3):
                    nc.tensor.matmul(
                        ps[:, off : off + OW],
                        lhsT=ctxT_sb[p][:, tt * TS : (tt + 1) * TS],
                        rhs=wo_sl(p, oh),
                        start=(p == 0),
                        stop=False,
                        skip_group_check=True,
                    )
            for tt, oh, ps, off in splits:
                nc.tensor.matmul(
                    ps[:, off : off + OW],
                    lhsT=ctxT_sb[3][:, tt * TS : (tt + 1) * TS],
                    rhs=wo_sl(3, oh),
                    start=False,
                    stop=True,
                    skip_group_check=True,
                )
                out_drain(tt, oh, ps[:, off : off + OW])
            for tt in (t0u + 2, t0u + 3):
                out_proj(tt, 1)

    nc.compile()
    return nc


def make_tri_keep(ts=TS):
    """[128, 2*ts] duplicated keep-mask: cell (s, t) = 0 iff s > t else 1."""
    s = np.arange(128)[:, None]
    t = np.arange(ts)[None, :]
    tri = np.where(s > t, 0.0, 1.0).astype(np.float32)
    return np.concatenate([tri, tri], axis=1)


def group_rows(a, cols_slice=None):
    """[(G*128), W] -> [128, G*W]: row p holds the concat over groups g of
    a[g*128+p, :], so each SBUF tile partition is one contiguous DMA line."""
    g = a.shape[0] // 128
    if cols_slice is not None:
        a = a[:, cols_slice]
    return np.ascontiguousarray(
        a.reshape(g, 128, a.shape[1]).transpose(1, 0, 2).reshape(128, -1)
    )


def make_core_inputs(x_b, W_qkv, b_qkv, W_out, heads, C_sz=C, T_sz=T):
    """Build the per-core input map (numpy, host-side)."""
    n_pairs = len(heads) // 2
    n_qk = 2 * n_pairs
    VW = len(heads) * DK
    xT = np.ascontiguousarray(x_b.T).astype(BF16)
    wqk = np.empty((C_sz, n_qk * 128), np.float32)
    bqk = np.empty((128, n_qk), np.float32)
    wv = np.empty((C_sz, VW), np.float32)
    bv = np.empty((1, VW), np.float32)
    wo = np.empty((n_pairs * 128, C_sz), np.float32)
    for p in range(n_pairs):
        hA, hB = heads[2 * p], heads[2 * p + 1]
        # q tile (scaled by 1/sqrt(dk)=1/8), k tile
        for half, h in ((0, hA), (1, hB)):
            r0 = h * 3 * DK
            wqk[:, 2 * p * 128 + half * 64 : 2 * p * 128 + half * 64 + 64] = (
                W_qkv[r0 : r0 + DK].T / math.sqrt(DK)
            )
            bqk[half * 64 : half * 64 + 64, 2 * p] = b_qkv[r0 : r0 + DK] / math.sqrt(DK)
            wqk[:, (2 * p + 1) * 128 + half * 64 : (2 * p + 1) * 128 + half * 64 + 64] = (
                W_qkv[r0 + DK : r0 + 2 * DK].T
            )
            bqk[half * 64 : half * 64 + 64, 2 * p + 1] = b_qkv[r0 + DK : r0 + 2 * DK]
            wo[p * 128 + half * 64 : p * 128 + half * 64 + 64, :] = W_out[
                :, h * DK : (h + 1) * DK
            ].T
    for hh, h in enumerate(heads):
        r0 = h * 3 * DK + 2 * DK
        wv[:, hh * DK : (hh + 1) * DK] = W_qkv[r0 : r0 + DK].T
        bv[0, hh * DK : (hh + 1) * DK] = b_qkv[r0 : r0 + DK]
    return {
        "xTa": group_rows(xT, np.s_[0:TSL]),
        "xTb": group_rows(xT, np.s_[TSL:T_sz]),
        "wqkT": group_rows(wqk.astype(BF16)),
        "wvT": group_rows(wv.astype(BF16)),
        "bqk": bqk.astype(np.float32),
        "bv": bv.astype(BF16),
        "woT": group_rows(wo.astype(BF16)),
        "masksq": make_tri_keep().astype(BF16),
    }


_NC_CACHE = {}


def kernel(x, W_qkv, b_qkv, W_out, b_out, _trace=False):
    x = np.asarray(x, dtype=np.float32)
    W_qkv = np.asarray(W_qkv, dtype=np.float32)
    b_qkv = np.asarray(b_qkv, dtype=np.float32)
    W_out = np.asarray(W_out, dtype=np.float32)
    b_out = np.asarray(b_out, dtype=np.float32)

    from concourse.bass_utils import run_bass_kernel_spmd

    key = ("full", C, T, 4)
    if key not in _NC_CACHE:
        _NC_CACHE[key] = build_program(C, T, n_pairs=4, num_devices=1)
    nc = _NC_CACHE[key]

    in_maps = []
    for core in range(NCORES):
        b, hg = divmod(core, 2)
        heads = list(range(hg * 8, hg * 8 + 8))
        in_maps.append(make_core_inputs(x[b], W_qkv, b_qkv, W_out, heads))

    res = run_bass_kernel_spmd(nc, in_maps, list(range(NCORES)), trace=_trace)
    kernel._last_results = res

    out = np.broadcast_to(b_out, (B, T, C)).astype(np.float32).copy()
    for core in range(NCORES):
        b = core // 2
        out[b] += res.results[core]["out"]
    return out


# revision 24
# speedup vs baseline: 1.1961x; 1.0014x over previous
"""Causal self-attention Trainium2 kernel (B=4, T=2048, D=1024, H=16).

Sharding: 8 cores = 4 batches x 2 head-groups (8 heads each). Each core
computes its batch's qkv projection restricted to its 8 heads, causal
attention for those heads, and a partial out-projection over its 512 ctx
channels. Host sums the two partials per batch and adds b_out.

Per-core layout choices (all matmuls bf16 with fp32 PSUM accumulation):
  - xT [C, T]: channels on partitions (contraction dim for projections).
    Split into a [C, 512] head tile and [C, 1536] tail tile so the first
    projections only wait on ~2MB of DMA.
  - qkT: per head-pair p, a q-tile [128, T] (head A rows 0:64, head B rows
    64:128) and a k-tile [128, T]. Produced directly transposed by making
    W the stationary operand. The 1/sqrt(dk) scale is folded into Wq/bq.
  - scoresT[s, t] blocks [128, 512]: lhsT=kT (K=64 rows), rhs=qT. Heads A/B
    are row-packed (tile_position rows 0:64 / 64:128) and run concurrently.
    Diagonal blocks only compute the causally needed t-range.
  - causal mask: diagonal 128x128 squares are zeroed AFTER exp by one DVE
    multiply with a 0/1 triangle (raw scores are bounded, so exp cannot
    overflow bf16); cheaper than the former identity-matmul -30000
    accumulation on the PE.
  - softmax: no max-subtraction (scores are within +-10 by construction),
    exp on ScalarE PSUM->SBUF bf16. ScalarE runs ONLY exp: the qk bias-add
    and out-proj PSUM drain live on DVE so the exp stream is never stalled
    behind interleaved IDENTITY ops (measured: ScalarE copies delay exp and
    cost ~50us end-to-end).
  - ctx: v stored naturally [s, d] with a ones column appended per head
    (v_ext [128, 8*65]); lhsT=v_ext (M=65) so PSUM row 64 accumulates the
    softmax denominator. Normalize = reciprocal_approx_fast + gpsimd
    partition_broadcast + DVE mul into the bf16 ctxT copy.
  - out projection: ctxT pair-tiles [128, T] are the stationary operand
    against W_outT; b_out is added on the host (once per batch).

Scheduling: attention blocks are exp-gated (ScalarE ~985ns/block vs PE
~640ns/block), so projection / out-projection units are spread as
"fillers" BETWEEN attention j-blocks (Bresenham; back-weighted in the
last iteration where the exp backlog peaks) to keep the PE busy while
exp catches up. The attention inner loop is software-pipelined by one
block (ctx(j-1) emitted after scores(j)) so the in-order PE queue never
head-blocks on exp. The tail out-projection emits its p=0..2
accumulation chains before the final pair's normalize so the PE stays
warm through it. Filler map: i=0: qk/v(1); i=1: qk/v(2)+out(0);
i=2: qk/v(3); i=3: out(1)+out(2); tail: out(3). PSUM: scores 2x2 banks,
ctx 2, proj/out 2. Inputs are host-packed p-major so every weight/x
tile is one contiguous 8KB-per-line DMA.

Measured on trn2: 285us (baseline of this design before scheduling work:
305us). Engine busy: PE ~245us (floor ~196us), ScalarE ~152us (exp),
DVE ~136us. Fixed overheads: ~7us SPMD preamble + ~10us input DMA +
~6us exit barrier.
"""

import math

import numpy as np
import ml_dtypes

B, T, C = 4, 2048, 1024
H, DK = 16, 64
NCORES = 8
TS = 128  # s-tile (partition granularity)
TSL = 512  # t free-dim tile (one PSUM bank of fp32)
MASK_VAL = -30000.0
BF16 = ml_dtypes.bfloat16


def build_program(C_sz=C, T_sz=T, n_pairs=4, num_devices=1):
    import concourse.mybir as mybir
    from concourse import bacc
    from concourse.tile import TileContext

    dt = mybir.dt
    f32 = dt.float32
    bf16 = dt.bfloat16
    AF = mybir.ActivationFunctionType

    n_ct = C_sz // 128  # contraction tiles for projections
    n_qk = 2 * n_pairs  # qk o-tiles (128 channels each)
    VW = n_pairs * 2 * DK  # v channels (natural order)
    n_tt = T_sz // TS
    n_it = T_sz // TSL
    JPI = TSL // TS  # s-tiles per i-tile (4)
    OW = min(TSL, C_sz)  # output column tile width
    n_oh = C_sz // OW  # output column halves
    VEW = n_pairs * 2 * (DK + 1)  # v_ext width (65 per head)

    nc = bacc.Bacc(
        "TRN2",
        target_bir_lowering=False,
        debug=False,
        num_devices=num_devices,
    )

    # All large inputs are pre-arranged on the host so each SBUF tile is one
    # contiguous [128, W] DMA (8KB+ per-partition lines; strided gathers from
    # DRAM are descriptor-bound and ~3x slower).
    XBW = T_sz - TSL
    xTa_d = nc.dram_tensor("xTa", [128, n_ct * TSL], bf16, kind="ExternalInput").ap()
    xTb_d = nc.dram_tensor("xTb", [128, n_ct * XBW], bf16, kind="ExternalInput").ap()
    wqk_d = nc.dram_tensor(
        "wqkT", [128, n_ct * n_qk * 128], bf16, kind="ExternalInput"
    ).ap()
    wv_d = nc.dram_tensor("wvT", [128, n_ct * VW], bf16, kind="ExternalInput").ap()
    bqk_d = nc.dram_tensor("bqk", [128, n_qk], f32, kind="ExternalInput").ap()
    bv_d = nc.dram_tensor("bv", [1, VW], bf16, kind="ExternalInput").ap()
    wo_d = nc.dram_tensor(
        "woT", [128, n_pairs * C_sz], bf16, kind="ExternalInput"
    ).ap()
    mask_d = nc.dram_tensor("masksq", [128, 2 * TS], bf16, kind="ExternalInput").ap()
    out_d = nc.dram_tensor("out", [T_sz, C_sz], f32, kind="ExternalOutput").ap()

    with TileContext(nc) as tc:
        with (
            tc.tile_pool(name="const", bufs=1) as const_pool,
            tc.tile_pool(name="big", bufs=1) as big_pool,
            tc.tile_pool(name="attn", bufs=10) as attn_pool,
            tc.tile_pool(name="rinv", bufs=6) as rinv_pool,
            tc.tile_pool(name="rbc", bufs=6) as rbc_pool,
            tc.tile_pool(name="outsb", bufs=6) as outsb_pool,
            tc.tile_pool(name="sc", bufs=2, space="PSUM") as sc_ps,
            tc.tile_pool(name="ctx", bufs=2, space="PSUM") as ctx_ps,
            tc.tile_pool(name="mm", bufs=2, space="PSUM") as mm_ps,
        ):
            # ---- input loads: one contiguous DMA per tensor, ordered so
            # the first matmuls start ASAP. v_proj(tt<4) needs wv +
            # xT[:, 0:512]; qk_proj(*, 0) additionally needs wqk. The tri
            # mask is needed by the (all-diagonal) i=0 attention blocks, so
            # it comes before the xT tail.
            def load_flat(name, dram_ap, cols):
                t = big_pool.tile([128, cols], bf16, tag=name, name=name)
                nc.sync.dma_start(t[:], dram_ap)
                return t

            wv_all = load_flat("wv", wv_d, n_ct * VW)
            xTa_all = load_flat("xTa", xTa_d, n_ct * TSL)
            bv_sb = const_pool.tile([1, VW], bf16, tag="bv", name="bv")
            nc.sync.dma_start(bv_sb[:], bv_d)
            bv_bc = const_pool.tile([128, VW], bf16, tag="bv_bc", name="bv_bc")
            nc.gpsimd.partition_broadcast(bv_bc[:], bv_sb[:])
            bqk_sb = const_pool.tile([128, n_qk], f32, tag="bqk", name="bqk")
            nc.sync.dma_start(bqk_sb[:], bqk_d)
            tri_sb = const_pool.tile([128, 2 * TS], bf16, tag="tri", name="tri")
            nc.sync.dma_start(tri_sb[:], mask_d)
            wqk_all = load_flat("wqk", wqk_d, n_ct * n_qk * 128)
            xTb_all = load_flat("xTb", xTb_d, n_ct * XBW)
            wo_all = load_flat("wo", wo_d, n_pairs * C_sz)

            def wv_sl(ci):
                return wv_all[:, ci * VW : (ci + 1) * VW]

            def wqk_sl(ci, ot):
                b = ci * n_qk * 128 + ot * 128
                return wqk_all[:, b : b + 128]

            def wo_sl(p, oh):
                b = p * C_sz + oh * OW
                return wo_all[:, b : b + OW]

            qkT_sb = [
                big_pool.tile([128, T_sz], bf16, tag=f"qkT{ot}", name=f"qkT{ot}")
                for ot in range(n_qk)
            ]
            vext_sb = [
                big_pool.tile([128, VEW], bf16, tag=f"vext{tt}", name=f"vext{tt}")
                for tt in range(n_tt)
            ]
            ctxT_sb = [
                big_pool.tile([128, T_sz], bf16, tag=f"ctxT{p}", name=f"ctxT{p}")
                for p in range(n_pairs)
            ]

            def x_cols(c0, c1):
                """AP for xT columns [c0:c1) of contraction tile ci."""
                if c1 <= TSL:
                    return lambda ci: xTa_all[:, ci * TSL + c0 : ci * TSL + c1]
                return lambda ci: xTb_all[
                    :, ci * XBW + c0 - TSL : ci * XBW + c1 - TSL
                ]

            def qk_proj(ot, i):
                xs = x_cols(i * TSL, (i + 1) * TSL)
                ps = mm_ps.tile([128, TSL], f32, tag="mm", name="mm")
                for ci in range(n_ct):
                    nc.tensor.matmul(
                        ps[:],
                        lhsT=wqk_sl(ci, ot),
                        rhs=xs(ci),
                        start=(ci == 0),
                        stop=(ci == n_ct - 1),
                    )
                nc.vector.tensor_scalar_add(
                    qkT_sb[ot][:, i * TSL : (i + 1) * TSL],
                    ps[:],
                    bqk_sb[:, ot : ot + 1],
                )

            def v_proj(tt):
                xs = x_cols(tt * TS, (tt + 1) * TS)
                ps = mm_ps.tile([128, VW], f32, tag="mm", name="mm")
                for ci in range(n_ct):
                    nc.tensor.matmul(
                        ps[:],
                        lhsT=xs(ci),
                        rhs=wv_sl(ci),
                        start=(ci == 0),
                        stop=(ci == n_ct - 1),
                    )
                vx = vext_sb[tt]
                vx3 = vx[:].rearrange("p (h e) -> p h e", e=DK + 1)
                nc.gpsimd.memset(vx3[:, :, DK : DK + 1], 1.0)
                nc.vector.scalar_tensor_tensor(
                    vx3[:, :, 0:DK],
                    ps[:].rearrange("p (h e) -> p h e", e=DK),
                    1.0,
                    bv_bc[:].rearrange("p (h e) -> p h e", e=DK),
                    op0=mybir.AluOpType.mult,
                    op1=mybir.AluOpType.add,
                )

            def out_proj(tt, oh):
                ps = mm_ps.tile([128, OW], f32, tag="mm", name="mm")
                for p in range(n_pairs):
                    nc.tensor.matmul(
                        ps[:],
                        lhsT=ctxT_sb[p][:, tt * TS : (tt + 1) * TS],
                        rhs=wo_sl(p, oh),
                        start=(p == 0),
                        stop=(p == n_pairs - 1),
                    )
                out_drain(tt, oh, ps[:])

            def out_drain(tt, oh, ps_ap):
                ob = outsb_pool.tile([128, OW], f32, tag="outsb", name="outsb")
                nc.vector.tensor_copy(ob[:], ps_ap)
                nc.sync.dma_start(
                    out_d[tt * TS : (tt + 1) * TS, oh * OW : (oh + 1) * OW],
                    ob[:],
                )

            def attn_pair(p, i, tick):
                qt, kt = qkT_sb[2 * p], qkT_sb[2 * p + 1]
                nj = JPI * (i + 1)
                ctxA = ctx_ps.tile([DK + 1, TSL], f32, tag="ctx", name="ctx")
                ctxB = ctx_ps.tile([DK + 1, TSL], f32, tag="ctx", name="ctx")

                def block_t0(j):
                    return (j - JPI * i) * TS if j >= JPI * i else 0

                def emit_ctx(j, a):
                    t0 = block_t0(j)
                    nc.tensor.matmul(
                        ctxA[:, t0:TSL],
                        lhsT=vext_sb[j][:, (2 * p) * (DK + 1) : (2 * p + 1) * (DK + 1)],
                        rhs=a[:, t0:TSL],
                        start=(j == 0),
                        stop=(j == nj - 1),
                    )
                    nc.tensor.matmul(
                        ctxB[:, t0:TSL],
                        lhsT=vext_sb[j][
                            :, (2 * p + 1) * (DK + 1) : (2 * p + 2) * (DK + 1)
                        ],
                        rhs=a[:, TSL + t0 : 2 * TSL],
                        start=(j == 0),
                        stop=(j == nj - 1),
                    )
                    tick()

                # One-block software pipeline: ctx(j-1) is emitted after
                # scores(j), so the in-order PE queue never heads-of-line
                # blocks on exp(j) — by the time ctx(j-1) issues, its exp
                # finished during scores(j).
                a_prev = None
                for j in range(nj):
                    diag = j >= JPI * i
                    t0 = block_t0(j)
                    ps = sc_ps.tile([128, 2 * TSL], f32, tag="sc", name="sc")
                    nc.tensor.matmul(
                        ps[:, t0:TSL],
                        lhsT=kt[0:64, j * TS : (j + 1) * TS],
                        rhs=qt[0:64, i * TSL + t0 : (i + 1) * TSL],
                        start=True,
                        stop=True,
                        skip_group_check=True,
                    )
                    nc.tensor.matmul(
                        ps[:, TSL + t0 : 2 * TSL],
                        lhsT=kt[64:128, j * TS : (j + 1) * TS],
                        rhs=qt[64:128, i * TSL + t0 : (i + 1) * TSL],
                        start=True,
                        stop=True,
                        skip_group_check=True,
                    )
                    a = attn_pool.tile([128, 2 * TSL], bf16, tag="attn", name="attn")
                    a3 = a[:].rearrange("p (c w) -> p c w", c=2)
                    ps3 = ps[:].rearrange("p (c w) -> p c w", c=2)
                    nc.scalar.activation(a3[:, :, t0:TSL], ps3[:, :, t0:TSL], AF.Exp)
                    if diag:
                        # zero the causally-dead triangle of the diagonal
                        # square (cheaper on DVE than the former identity
                        # matmul accumulating -30000 into PSUM; raw scores
                        # are bounded so exp can't overflow bf16).
                        nc.vector.tensor_tensor(
                            a3[:, :, t0 : t0 + TS],
                            a3[:, :, t0 : t0 + TS],
                            tri_sb[:].rearrange("p (c w) -> p c w", c=2),
                            op=mybir.AluOpType.mult,
                        )
                    if a_prev is not None:
                        emit_ctx(j - 1, a_prev)
                    a_prev = a
                emit_ctx(nj - 1, a_prev)
                isl = slice(i * TSL, (i + 1) * TSL)
                # A/B chains interleaved so the gpsimd broadcast of head A
                # overlaps the DVE copy/recip of head B (shorter critical
                # path into the out-projection that consumes ctxT).
                # custom-DVE ops misread PSUM on hw: bounce rowsum via SBUF.
                rcps = []
                for cps in (ctxA, ctxB):
                    rs = rinv_pool.tile([1, TSL], f32, tag="rsum", name="rsum")
                    nc.vector.tensor_copy(rs[:], cps[DK : DK + 1, :])
                    r = rinv_pool.tile([1, TSL], f32, tag="rinv", name="rinv")
                    nc.vector.reciprocal_approx_fast(r[:], rs[:])
                    rcps.append(r)
                for cps, rows, r in (
                    (ctxA, slice(0, 64), rcps[0]),
                    (ctxB, slice(64, 128), rcps[1]),
                ):
                    rbc = rbc_pool.tile([DK, TSL], f32, tag="rbc", name="rbc")
                    nc.gpsimd.partition_broadcast(rbc[:], r[:])
                    nc.vector.tensor_mul(ctxT_sb[p][rows, isl], cps[0:DK, :], rbc[:])

            # ---- software-pipelined emission ----
            for tt in range(0, JPI):
                v_proj(tt)
            for ot in range(n_qk):
                qk_proj(ot, 0)

            def filler_units(i):
                F = []
                if i + 1 < n_it:
                    qk = [lambda ot=ot: qk_proj(ot, i + 1) for ot in range(n_qk)]
                    vv = [
                        lambda tt=tt: v_proj(tt)
                        for tt in range(JPI * (i + 1), JPI * (i + 2))
                    ]
                    for a in range(JPI):
                        F.append(qk[2 * a])
                        F.append(qk[2 * a + 1])
                        F.append(vv[a])
                # out-proj batches are assigned to exp-heavy iterations:
                # out(0) during i=1, out(1)+out(2) during i=3, out(3) at end.
                outs = {1: [0], 3: [1, 2]}.get(i, [])
                for io in outs:
                    for tt in range(JPI * io, JPI * (io + 1)):
                        for oh in range(n_oh):
                            F.append(lambda tt=tt, oh=oh: out_proj(tt, oh))
                return F

            for i in range(n_it):
                F = filler_units(i)
                NB = n_pairs * JPI * (i + 1)
                nb = 0
                nf = 0

                def tick():
                    nonlocal nb, nf
                    nb += 1
                    if i == n_it - 1:
                        # back-weighted spread: ScalarE's exp backlog peaks at
                        # the end of the last iteration, so save fillers for it
                        want = len(F) * nb * nb // (NB * NB)
                    else:
                        want = len(F) * nb // NB
                    while nf < want:
                        F[nf]()
                        nf += 1

                for p in range(n_pairs):
                    attn_pair(p, i, tick)
                while nf < len(F):
                    F[nf]()
                    nf += 1

            # ---- tail: out-proj of the last i-block. The final pair's
            # normalize chain (~4us on DVE/gpsimd) gates only the p=3
            # matmuls, so emit the p=0..2 accumulation chains first — they
            # execute during the normalize, keeping the PE busy and out of
            # the low p-state. 6 of 8 units get persistent PSUM (2 sc tiles
            # hold two [128,OW] halves each + 2 mm tiles); the last 2 run
            # as ordinary units.
            t0u = JPI * (n_it - 1)
            sc1 = sc_ps.tile([128, 2 * TSL], f32, tag="sc", name="sc")
            sc2 = sc_ps.tile([128, 2 * TSL], f32, tag="sc", name="sc")
            splits = [
                (t0u + 0, 0, sc1, 0),
                (t0u + 1, 0, sc1, TSL),
                (t0u + 2, 0, sc2, 0),
                (t0u + 3, 0, sc2, TSL),
                (t0u + 0, 1, mm_ps.tile([128, OW], f32, tag="mm", name="mm"), 0),
                (t0u + 1, 1, mm_ps.tile([128, OW], f32, tag="mm", name="mm"), 0),
            ]
            for tt, oh, ps, off in splits:
                for p in range(3):
                    nc.tensor.matmul(
                        ps[:, off : off + OW],
                        lhsT=ctxT_sb[p][:, tt * TS : (tt + 1) * TS],
                        rhs=wo_sl(p, oh),
                        start=(p == 0),
                        stop=False,
                        skip_group_check=True,
                    )
            for tt, oh, ps, off in splits:
                nc.tensor.matmul(
                    ps[:, off : off + OW],
                    lhsT=ctxT_sb[3][:, tt * TS : (tt + 1) * TS],
                    rhs=wo_sl(3, oh),
                    start=False,
                    stop=True,
                    skip_group_check=True,
                )
                out_drain(tt, oh, ps[:, off : off + OW])
            for tt in (t0u + 2, t0u + 3):
                out_proj(tt, 1)

    nc.compile()
    return nc


def make_tri_keep(ts=TS):
    """[128, 2*ts] duplicated keep-mask: cell (s, t) = 0 iff s > t else 1."""
    s = np.arange(128)[:, None]
    t = np.arange(ts)[None, :]
    tri = np.where(s > t, 0.0, 1.0).astype(np.float32)
    return np.concatenate([tri, tri], axis=1)


def group_rows(a, cols_slice=None):
    """[(G*128), W] -> [128, G*W]: row p holds the concat over groups g of
    a[g*128+p, :], so each SBUF tile partition is one contiguous DMA line."""
    g = a.shape[0] // 128
    if cols_slice is not None:
        a = a[:, cols_slice]
    return np.ascontiguousarray(
        a.reshape(g, 128, a.shape[1]).transpose(1, 0, 2).reshape(128, -1)
    )


def make_core_inputs(x_b, W_qkv, b_qkv, W_out, heads, C_sz=C, T_sz=T):
    """Build the per-core input map (numpy, host-side)."""
    n_pairs = len(heads) // 2
    n_qk = 2 * n_pairs
    VW = len(heads) * DK
    xT = np.ascontiguousarray(x_b.T).astype(BF16)
    wqk = np.empty((C_sz, n_qk * 128), np.float32)
    bqk = np.empty((128, n_qk), np.float32)
    wv = np.empty((C_sz, VW), np.float32)
    bv = np.empty((1, VW), np.float32)
    wo = np.empty((n_pairs * 128, C_sz), np.float32)
    for p in range(n_pairs):
        hA, hB = heads[2 * p], heads[2 * p + 1]
        # q tile (scaled by 1/sqrt(dk)=1/8), k tile
        for half, h in ((0, hA), (1, hB)):
            r0 = h * 3 * DK
            wqk[:, 2 * p * 128 + half * 64 : 2 * p * 128 + half * 64 + 64] = (
                W_qkv[r0 : r0 + DK].T / math.sqrt(DK)
            )
            bqk[half * 64 : half * 64 + 64, 2 * p] = b_qkv[r0 : r0 + DK] / math.sqrt(DK)
            wqk[:, (2 * p + 1) * 128 + half * 64 : (2 * p + 1) * 128 + half * 64 + 64] = (
                W_qkv[r0 + DK : r0 + 2 * DK].T
            )
            bqk[half * 64 : half * 64 + 64, 2 * p + 1] = b_qkv[r0 + DK : r0 + 2 * DK]
            wo[p * 128 + half * 64 : p * 128 + half * 64 + 64, :] = W_out[
                :, h * DK : (h + 1) * DK
            ].T
    for hh, h in enumerate(heads):
        r0 = h * 3 * DK + 2 * DK
        wv[:, hh * DK : (hh + 1) * DK] = W_qkv[r0 : r0 + DK].T
        bv[0, hh * DK : (hh + 1) * DK] = b_qkv[r0 : r0 + DK]
    return {
        "xTa": group_rows(xT, np.s_[0:TSL]),
        "xTb": group_rows(xT, np.s_[TSL:T_sz]),
        "wqkT": group_rows(wqk.astype(BF16)),
        "wvT": group_rows(wv.astype(BF16)),
        "bqk": bqk.astype(np.float32),
        "bv": bv.astype(BF16),
        "woT": group_rows(wo.astype(BF16)),
        "masksq": make_tri_keep().astype(BF16),
    }


_NC_CACHE = {}


def kernel(x, W_qkv, b_qkv, W_out, b_out, _trace=False):
    x = np.asarray(x, dtype=np.float32)
    W_qkv = np.asarray(W_qkv, dtype=np.float32)
    b_qkv = np.asarray(b_qkv, dtype=np.float32)
    W_out = np.asarray(W_out, dtype=np.float32)
    b_out = np.asarray(b_out, dtype=np.float32)

    from concourse.bass_utils import run_bass_kernel_spmd

    key = ("full", C, T, 4)
    if key not in _NC_CACHE:
        _NC_CACHE[key] = build_program(C, T, n_pairs=4, num_devices=1)
    nc = _NC_CACHE[key]

    in_maps = []
    for core in range(NCORES):
        b, hg = divmod(core, 2)
        heads = list(range(hg * 8, hg * 8 + 8))
        in_maps.append(make_core_inputs(x[b], W_qkv, b_qkv, W_out, heads))

    res = run_bass_kernel_spmd(nc, in_maps, list(range(NCORES)), trace=_trace)
    kernel._last_results = res

    out = np.broadcast_to(b_out, (B, T, C)).astype(np.float32).copy()
    for core in range(NCORES):
        b = core // 2
        out[b] += res.results[core]["out"]
    return out


# revision 25
# speedup vs baseline: 1.2009x; 1.0041x over previous
"""Causal self-attention Trainium2 kernel (B=4, T=2048, D=1024, H=16).

Sharding: 8 cores = 4 batches x 2 head-groups (8 heads each). Each core
computes its batch's qkv projection restricted to its 8 heads, causal
attention for those heads, and a partial out-projection over its 512 ctx
channels. Host sums the two partials per batch and adds b_out.

Per-core layout choices (all matmuls bf16 with fp32 PSUM accumulation):
  - xT [C, T]: channels on partitions (contraction dim for projections).
    Split into a [C, 512] head tile and [C, 1536] tail tile so the first
    projections only wait on ~2MB of DMA.
  - qkT: per head-pair p, a q-tile [128, T] (head A rows 0:64, head B rows
    64:128) and a k-tile [128, T]. Produced directly transposed by making
    W the stationary operand. The 1/sqrt(dk) scale is folded into Wq/bq.
  - scoresT[s, t] blocks [128, 512]: lhsT=kT (K=64 rows), rhs=qT. Heads A/B
    are row-packed (tile_position rows 0:64 / 64:128) and run concurrently.
    Diagonal blocks only compute the causally needed t-range.
  - causal mask: diagonal 128x128 squares are zeroed AFTER exp by one DVE
    multiply with a 0/1 triangle (raw scores are bounded, so exp cannot
    overflow bf16); cheaper than the former identity-matmul -30000
    accumulation on the PE.
  - softmax: no max-subtraction (scores are within +-10 by construction),
    exp on ScalarE PSUM->SBUF bf16. ScalarE runs ONLY exp: the qk bias-add
    and out-proj PSUM drain live on DVE so the exp stream is never stalled
    behind interleaved IDENTITY ops (measured: ScalarE copies delay exp and
    cost ~50us end-to-end).
  - ctx: v stored naturally [s, d] with a ones column appended per head
    (v_ext [128, 8*65]); lhsT=v_ext (M=65) so PSUM row 64 accumulates the
    softmax denominator. Normalize = reciprocal_approx_fast + gpsimd
    partition_broadcast + DVE mul into the bf16 ctxT copy.
  - out projection: ctxT pair-tiles [128, T] are the stationary operand
    against W_outT; b_out is added on the host (once per batch).

Scheduling: attention blocks are exp-gated (ScalarE ~985ns/block vs PE
~640ns/block), so projection / out-projection units are spread as
"fillers" BETWEEN attention j-blocks (Bresenham; back-weighted in the
last iteration where the exp backlog peaks) to keep the PE busy while
exp catches up. The attention inner loop is software-pipelined by one
block (ctx(j-1) emitted after scores(j)) so the in-order PE queue never
head-blocks on exp. The tail out-projection emits its p=0..2
accumulation chains before the final pair's normalize so the PE stays
warm through it. Filler map: i=0: qk/v(1); i=1: qk/v(2)+out(0);
i=2: qk/v(3); i=3: out(1)+out(2); tail: out(3). PSUM: scores 2x2 banks,
ctx 2, proj/out 2. Inputs are host-packed p-major so every weight/x
tile is one contiguous 8KB-per-line DMA.

Measured on trn2: 285us (baseline of this design before scheduling work:
305us). Engine busy: PE ~245us (floor ~196us), ScalarE ~152us (exp),
DVE ~136us. Fixed overheads: ~7us SPMD preamble + ~10us input DMA +
~6us exit barrier.
"""

import math

import numpy as np
import ml_dtypes

B, T, C = 4, 2048, 1024
H, DK = 16, 64
NCORES = 8
TS = 128  # s-tile (partition granularity)
TSL = 512  # t free-dim tile (one PSUM bank of fp32)
MASK_VAL = -30000.0
BF16 = ml_dtypes.bfloat16


def build_program(C_sz=C, T_sz=T, n_pairs=4, num_devices=1):
    import concourse.mybir as mybir
    from concourse import bacc
    from concourse.tile import TileContext

    dt = mybir.dt
    f32 = dt.float32
    bf16 = dt.bfloat16
    AF = mybir.ActivationFunctionType

    n_ct = C_sz // 128  # contraction tiles for projections
    n_qk = 2 * n_pairs  # qk o-tiles (128 channels each)
    VW = n_pairs * 2 * DK  # v channels (natural order)
    n_tt = T_sz // TS
    n_it = T_sz // TSL
    JPI = TSL // TS  # s-tiles per i-tile (4)
    OW = min(TSL, C_sz)  # output column tile width
    n_oh = C_sz // OW  # output column halves
    VEW = n_pairs * 2 * (DK + 1)  # v_ext width (65 per head)

    nc = bacc.Bacc(
        "TRN2",
        target_bir_lowering=False,
        debug=False,
        num_devices=num_devices,
    )

    # All large inputs are pre-arranged on the host so each SBUF tile is one
    # contiguous [128, W] DMA (8KB+ per-partition lines; strided gathers from
    # DRAM are descriptor-bound and ~3x slower).
    XBW = T_sz - TSL
    xTa_d = nc.dram_tensor("xTa", [128, n_ct * TSL], bf16, kind="ExternalInput").ap()
    xTb_d = nc.dram_tensor("xTb", [128, n_ct * XBW], bf16, kind="ExternalInput").ap()
    wqk_d = nc.dram_tensor(
        "wqkT", [128, n_ct * n_qk * 128], bf16, kind="ExternalInput"
    ).ap()
    wv_d = nc.dram_tensor("wvT", [128, n_ct * VW], bf16, kind="ExternalInput").ap()
    bqk_d = nc.dram_tensor("bqk", [128, n_qk], f32, kind="ExternalInput").ap()
    bv_d = nc.dram_tensor("bv", [1, VW], bf16, kind="ExternalInput").ap()
    wo_d = nc.dram_tensor(
        "woT", [128, n_pairs * C_sz], bf16, kind="ExternalInput"
    ).ap()
    mask_d = nc.dram_tensor("masksq", [128, 2 * TS], bf16, kind="ExternalInput").ap()
    out_d = nc.dram_tensor("out", [T_sz, C_sz], f32, kind="ExternalOutput").ap()

    with TileContext(nc) as tc:
        with (
            tc.tile_pool(name="const", bufs=1) as const_pool,
            tc.tile_pool(name="big", bufs=1) as big_pool,
            tc.tile_pool(name="attn", bufs=10) as attn_pool,
            tc.tile_pool(name="rinv", bufs=6) as rinv_pool,
            tc.tile_pool(name="rbc", bufs=6) as rbc_pool,
            tc.tile_pool(name="outsb", bufs=6) as outsb_pool,
            tc.tile_pool(name="sc", bufs=2, space="PSUM") as sc_ps,
            tc.tile_pool(name="ctx", bufs=2, space="PSUM") as ctx_ps,
            tc.tile_pool(name="mm", bufs=2, space="PSUM") as mm_ps,
        ):
            # ---- input loads: one contiguous DMA per tensor, ordered so
            # the first matmuls start ASAP. v_proj(tt<4) needs wv +
            # xT[:, 0:512]; qk_proj(*, 0) additionally needs wqk. The tri
            # mask is needed by the (all-diagonal) i=0 attention blocks, so
            # it comes before the xT tail.
            def load_flat(name, dram_ap, cols):
                t = big_pool.tile([128, cols], bf16, tag=name, name=name)
                nc.sync.dma_start(t[:], dram_ap)
                return t

            # wv/xTa arrive as interleaved quarter-chunks: the first v_proj
            # accumulation steps (ci-ordered) only need the leading columns,
            # so compute starts after ~0.5MB instead of waiting for the full
            # 2MB to finish behind 6MB of other queued transfers — and the
            # v_proj chains warm the PE p-state before qk_proj.
            wv_all = big_pool.tile([128, n_ct * VW], bf16, tag="wv", name="wv")
            xTa_all = big_pool.tile([128, n_ct * TSL], bf16, tag="xTa", name="xTa")
            wv_step = n_ct * VW // 4
            xa_step = n_ct * TSL // 4
            for k in range(4):
                nc.sync.dma_start(
                    wv_all[:, k * wv_step : (k + 1) * wv_step],
                    wv_d[:, k * wv_step : (k + 1) * wv_step],
                )
                nc.sync.dma_start(
                    xTa_all[:, k * xa_step : (k + 1) * xa_step],
                    xTa_d[:, k * xa_step : (k + 1) * xa_step],
                )
            bv_sb = const_pool.tile([1, VW], bf16, tag="bv", name="bv")
            nc.sync.dma_start(bv_sb[:], bv_d)
            bv_bc = const_pool.tile([128, VW], bf16, tag="bv_bc", name="bv_bc")
            nc.gpsimd.partition_broadcast(bv_bc[:], bv_sb[:])
            bqk_sb = const_pool.tile([128, n_qk], f32, tag="bqk", name="bqk")
            nc.sync.dma_start(bqk_sb[:], bqk_d)
            tri_sb = const_pool.tile([128, 2 * TS], bf16, tag="tri", name="tri")
            nc.sync.dma_start(tri_sb[:], mask_d)
            wqk_all = load_flat("wqk", wqk_d, n_ct * n_qk * 128)
            xTb_all = load_flat("xTb", xTb_d, n_ct * XBW)
            wo_all = load_flat("wo", wo_d, n_pairs * C_sz)

            def wv_sl(ci):
                return wv_all[:, ci * VW : (ci + 1) * VW]

            def wqk_sl(ci, ot):
                b = ci * n_qk * 128 + ot * 128
                return wqk_all[:, b : b + 128]

            def wo_sl(p, oh):
                b = p * C_sz + oh * OW
                return wo_all[:, b : b + OW]

            qkT_sb = [
                big_pool.tile([128, T_sz], bf16, tag=f"qkT{ot}", name=f"qkT{ot}")
                for ot in range(n_qk)
            ]
            vext_sb = [
                big_pool.tile([128, VEW], bf16, tag=f"vext{tt}", name=f"vext{tt}")
                for tt in range(n_tt)
            ]
            ctxT_sb = [
                big_pool.tile([128, T_sz], bf16, tag=f"ctxT{p}", name=f"ctxT{p}")
                for p in range(n_pairs)
            ]

            def x_cols(c0, c1):
                """AP for xT columns [c0:c1) of contraction tile ci."""
                if c1 <= TSL:
                    return lambda ci: xTa_all[:, ci * TSL + c0 : ci * TSL + c1]
                return lambda ci: xTb_all[
                    :, ci * XBW + c0 - TSL : ci * XBW + c1 - TSL
                ]

            def qk_proj(ot, i):
                xs = x_cols(i * TSL, (i + 1) * TSL)
                ps = mm_ps.tile([128, TSL], f32, tag="mm", name="mm")
                for ci in range(n_ct):
                    nc.tensor.matmul(
                        ps[:],
                        lhsT=wqk_sl(ci, ot),
                        rhs=xs(ci),
                        start=(ci == 0),
                        stop=(ci == n_ct - 1),
                    )
                nc.vector.tensor_scalar_add(
                    qkT_sb[ot][:, i * TSL : (i + 1) * TSL],
                    ps[:],
                    bqk_sb[:, ot : ot + 1],
                )

            def v_proj(tt):
                xs = x_cols(tt * TS, (tt + 1) * TS)
                ps = mm_ps.tile([128, VW], f32, tag="mm", name="mm")
                for ci in range(n_ct):
                    nc.tensor.matmul(
                        ps[:],
                        lhsT=xs(ci),
                        rhs=wv_sl(ci),
                        start=(ci == 0),
                        stop=(ci == n_ct - 1),
                    )
                vx = vext_sb[tt]
                vx3 = vx[:].rearrange("p (h e) -> p h e", e=DK + 1)
                nc.gpsimd.memset(vx3[:, :, DK : DK + 1], 1.0)
                nc.vector.scalar_tensor_tensor(
                    vx3[:, :, 0:DK],
                    ps[:].rearrange("p (h e) -> p h e", e=DK),
                    1.0,
                    bv_bc[:].rearrange("p (h e) -> p h e", e=DK),
                    op0=mybir.AluOpType.mult,
                    op1=mybir.AluOpType.add,
                )

            def out_proj(tt, oh):
                ps = mm_ps.tile([128, OW], f32, tag="mm", name="mm")
                for p in range(n_pairs):
                    nc.tensor.matmul(
                        ps[:],
                        lhsT=ctxT_sb[p][:, tt * TS : (tt + 1) * TS],
                        rhs=wo_sl(p, oh),
                        start=(p == 0),
                        stop=(p == n_pairs - 1),
                    )
                out_drain(tt, oh, ps[:])

            def out_drain(tt, oh, ps_ap):
                ob = outsb_pool.tile([128, OW], f32, tag="outsb", name="outsb")
                nc.vector.tensor_copy(ob[:], ps_ap)
                nc.sync.dma_start(
                    out_d[tt * TS : (tt + 1) * TS, oh * OW : (oh + 1) * OW],
                    ob[:],
                )

            def attn_pair(p, i, tick):
                qt, kt = qkT_sb[2 * p], qkT_sb[2 * p + 1]
                nj = JPI * (i + 1)
                ctxA = ctx_ps.tile([DK + 1, TSL], f32, tag="ctx", name="ctx")
                ctxB = ctx_ps.tile([DK + 1, TSL], f32, tag="ctx", name="ctx")

                def block_t0(j):
                    return (j - JPI * i) * TS if j >= JPI * i else 0

                def emit_ctx(j, a):
                    t0 = block_t0(j)
                    nc.tensor.matmul(
                        ctxA[:, t0:TSL],
                        lhsT=vext_sb[j][:, (2 * p) * (DK + 1) : (2 * p + 1) * (DK + 1)],
                        rhs=a[:, t0:TSL],
                        start=(j == 0),
                        stop=(j == nj - 1),
                    )
                    nc.tensor.matmul(
                        ctxB[:, t0:TSL],
                        lhsT=vext_sb[j][
                            :, (2 * p + 1) * (DK + 1) : (2 * p + 2) * (DK + 1)
                        ],
                        rhs=a[:, TSL + t0 : 2 * TSL],
                        start=(j == 0),
                        stop=(j == nj - 1),
                    )
                    tick()

                # One-block software pipeline: ctx(j-1) is emitted after
                # scores(j), so the in-order PE queue never heads-of-line
                # blocks on exp(j) — by the time ctx(j-1) issues, its exp
                # finished during scores(j).
                a_prev = None
                for j in range(nj):
                    diag = j >= JPI * i
                    t0 = block_t0(j)
                    ps = sc_ps.tile([128, 2 * TSL], f32, tag="sc", name="sc")
                    nc.tensor.matmul(
                        ps[:, t0:TSL],
                        lhsT=kt[0:64, j * TS : (j + 1) * TS],
                        rhs=qt[0:64, i * TSL + t0 : (i + 1) * TSL],
                        start=True,
                        stop=True,
                        skip_group_check=True,
                    )
                    nc.tensor.matmul(
                        ps[:, TSL + t0 : 2 * TSL],
                        lhsT=kt[64:128, j * TS : (j + 1) * TS],
                        rhs=qt[64:128, i * TSL + t0 : (i + 1) * TSL],
                        start=True,
                        stop=True,
                        skip_group_check=True,
                    )
                    a = attn_pool.tile([128, 2 * TSL], bf16, tag="attn", name="attn")
                    a3 = a[:].rearrange("p (c w) -> p c w", c=2)
                    ps3 = ps[:].rearrange("p (c w) -> p c w", c=2)
                    nc.scalar.activation(a3[:, :, t0:TSL], ps3[:, :, t0:TSL], AF.Exp)
                    if diag:
                        # zero the causally-dead triangle of the diagonal
                        # square (cheaper on DVE than the former identity
                        # matmul accumulating -30000 into PSUM; raw scores
                        # are bounded so exp can't overflow bf16).
                        nc.vector.tensor_tensor(
                            a3[:, :, t0 : t0 + TS],
                            a3[:, :, t0 : t0 + TS],
                            tri_sb[:].rearrange("p (c w) -> p c w", c=2),
                            op=mybir.AluOpType.mult,
                        )
                    if a_prev is not None:
                        emit_ctx(j - 1, a_prev)
                    a_prev = a
                emit_ctx(nj - 1, a_prev)
                isl = slice(i * TSL, (i + 1) * TSL)
                # A/B chains interleaved so the gpsimd broadcast of head A
                # overlaps the DVE copy/recip of head B (shorter critical
                # path into the out-projection that consumes ctxT).
                # custom-DVE ops misread PSUM on hw: bounce rowsum via SBUF.
                rcps = []
                for cps in (ctxA, ctxB):
                    rs = rinv_pool.tile([1, TSL], f32, tag="rsum", name="rsum")
                    nc.vector.tensor_copy(rs[:], cps[DK : DK + 1, :])
                    r = rinv_pool.tile([1, TSL], f32, tag="rinv", name="rinv")
                    nc.vector.reciprocal_approx_fast(r[:], rs[:])
                    rcps.append(r)
                for cps, rows, r in (
                    (ctxA, slice(0, 64), rcps[0]),
                    (ctxB, slice(64, 128), rcps[1]),
                ):
                    rbc = rbc_pool.tile([DK, TSL], f32, tag="rbc", name="rbc")
                    nc.gpsimd.partition_broadcast(rbc[:], r[:])
                    nc.vector.tensor_mul(ctxT_sb[p][rows, isl], cps[0:DK, :], rbc[:])

            # ---- software-pipelined emission ----
            for tt in range(0, JPI):
                v_proj(tt)
            for ot in range(n_qk):
                qk_proj(ot, 0)

            def filler_units(i):
                F = []
                if i + 1 < n_it:
                    qk = [lambda ot=ot: qk_proj(ot, i + 1) for ot in range(n_qk)]
                    vv = [
                        lambda tt=tt: v_proj(tt)
                        for tt in range(JPI * (i + 1), JPI * (i + 2))
                    ]
                    for a in range(JPI):
                        F.append(qk[2 * a])
                        F.append(qk[2 * a + 1])
                        F.append(vv[a])
                # out-proj batches are assigned to exp-heavy iterations:
                # out(0) during i=1, out(1)+out(2) during i=3, out(3) at end.
                outs = {1: [0], 3: [1, 2]}.get(i, [])
                for io in outs:
                    for tt in range(JPI * io, JPI * (io + 1)):
                        for oh in range(n_oh):
                            F.append(lambda tt=tt, oh=oh: out_proj(tt, oh))
                return F

            for i in range(n_it):
                F = filler_units(i)
                NB = n_pairs * JPI * (i + 1)
                nb = 0
                nf = 0

                def tick():
                    nonlocal nb, nf
                    nb += 1
                    if i == n_it - 1:
                        # back-weighted spread: ScalarE's exp backlog peaks at
                        # the end of the last iteration, so save fillers for it
                        want = len(F) * nb * nb // (NB * NB)
                    else:
                        want = len(F) * nb // NB
                    while nf < want:
                        F[nf]()
                        nf += 1

                for p in range(n_pairs):
                    attn_pair(p, i, tick)
                while nf < len(F):
                    F[nf]()
                    nf += 1

            # ---- tail: out-proj of the last i-block. The final pair's
            # normalize chain (~4us on DVE/gpsimd) gates only the p=3
            # matmuls, so emit the p=0..2 accumulation chains first — they
            # execute during the normalize, keeping the PE busy and out of
            # the low p-state. 6 of 8 units get persistent PSUM (2 sc tiles
            # hold two [128,OW] halves each + 2 mm tiles); the last 2 run
            # as ordinary units.
            t0u = JPI * (n_it - 1)
            sc1 = sc_ps.tile([128, 2 * TSL], f32, tag="sc", name="sc")
            sc2 = sc_ps.tile([128, 2 * TSL], f32, tag="sc", name="sc")
            splits = [
                (t0u + 0, 0, sc1, 0),
                (t0u + 1, 0, sc1, TSL),
                (t0u + 2, 0, sc2, 0),
                (t0u + 3, 0, sc2, TSL),
                (t0u + 0, 1, mm_ps.tile([128, OW], f32, tag="mm", name="mm"), 0),
                (t0u + 1, 1, mm_ps.tile([128, OW], f32, tag="mm", name="mm"), 0),
            ]
            for tt, oh, ps, off in splits:
                for p in range(3):
                    nc.tensor.matmul(
                        ps[:, off : off + OW],
                        lhsT=ctxT_sb[p][:, tt * TS : (tt + 1) * TS],
                        rhs=wo_sl(p, oh),
                        start=(p == 0),
                        stop=False,
                        skip_group_check=True,
                    )
            for tt, oh, ps, off in splits:
                nc.tensor.matmul(
                    ps[:, off : off + OW],
                    lhsT=ctxT_sb[3][:, tt * TS : (tt + 1) * TS],
                    rhs=wo_sl(3, oh),
                    start=False,
                    stop=True,
                    skip_group_check=True,
                )
                out_drain(tt, oh, ps[:, off : off + OW])
            for tt in (t0u + 2, t0u + 3):
                out_proj(tt, 1)

    nc.compile()
    return nc


def make_tri_keep(ts=TS):
    """[128, 2*ts] duplicated keep-mask: cell (s, t) = 0 iff s > t else 1."""
    s = np.arange(128)[:, None]
    t = np.arange(ts)[None, :]
    tri = np.where(s > t, 0.0, 1.0).astype(np.float32)
    return np.concatenate([tri, tri], axis=1)


def group_rows(a, cols_slice=None):
    """[(G*128), W] -> [128, G*W]: row p holds the concat over groups g of
    a[g*128+p, :], so each SBUF tile partition is one contiguous DMA line."""
    g = a.shape[0] // 128
    if cols_slice is not None:
        a = a[:, cols_slice]
    return np.ascontiguousarray(
        a.reshape(g, 128, a.shape[1]).transpose(1, 0, 2).reshape(128, -1)
    )


def make_core_inputs(x_b, W_qkv, b_qkv, W_out, heads, C_sz=C, T_sz=T):
    """Build the per-core input map (numpy, host-side)."""
    n_pairs = len(heads) // 2
    n_qk = 2 * n_pairs
    VW = len(heads) * DK
    xT = np.ascontiguousarray(x_b.T).astype(BF16)
    wqk = np.empty((C_sz, n_qk * 128), np.float32)
    bqk = np.empty((128, n_qk), np.float32)
    wv = np.empty((C_sz, VW), np.float32)
    bv = np.empty((1, VW), np.float32)
    wo = np.empty((n_pairs * 128, C_sz), np.float32)
    for p in range(n_pairs):
        hA, hB = heads[2 * p], heads[2 * p + 1]
        # q tile (scaled by 1/sqrt(dk)=1/8), k tile
        for half, h in ((0, hA), (1, hB)):
            r0 = h * 3 * DK
            wqk[:, 2 * p * 128 + half * 64 : 2 * p * 128 + half * 64 + 64] = (
                W_qkv[r0 : r0 + DK].T / math.sqrt(DK)
            )
            bqk[half * 64 : half * 64 + 64, 2 * p] = b_qkv[r0 : r0 + DK] / math.sqrt(DK)
            wqk[:, (2 * p + 1) * 128 + half * 64 : (2 * p + 1) * 128 + half * 64 + 64] = (
                W_qkv[r0 + DK : r0 + 2 * DK].T
            )
            bqk[half * 64 : half * 64 + 64, 2 * p + 1] = b_qkv[r0 + DK : r0 + 2 * DK]
            wo[p * 128 + half * 64 : p * 128 + half * 64 + 64, :] = W_out[
                :, h * DK : (h + 1) * DK
            ].T
    for hh, h in enumerate(heads):
        r0 = h * 3 * DK + 2 * DK
        wv[:, hh * DK : (hh + 1) * DK] = W_qkv[r0 : r0 + DK].T
        bv[0, hh * DK : (hh + 1) * DK] = b_qkv[r0 : r0 + DK]
    return {
        "xTa": group_rows(xT, np.s_[0:TSL]),
        "xTb": group_rows(xT, np.s_[TSL:T_sz]),
        "wqkT": group_rows(wqk.astype(BF16)),
        "wvT": group_rows(wv.astype(BF16)),
        "bqk": bqk.astype(np.float32),
        "bv": bv.astype(BF16),
        "woT": group_rows(wo.astype(BF16)),
        "masksq": make_tri_keep().astype(BF16),
    }


_NC_CACHE = {}


def kernel(x, W_qkv, b_qkv, W_out, b_out, _trace=False):
    x = np.asarray(x, dtype=np.float32)
    W_qkv = np.asarray(W_qkv, dtype=np.float32)
    b_qkv = np.asarray(b_qkv, dtype=np.float32)
    W_out = np.asarray(W_out, dtype=np.float32)
    b_out = np.asarray(b_out, dtype=np.float32)

    from concourse.bass_utils import run_bass_kernel_spmd

    key = ("full", C, T, 4)
    if key not in _NC_CACHE:
        _NC_CACHE[key] = build_program(C, T, n_pairs=4, num_devices=1)
    nc = _NC_CACHE[key]

    in_maps = []
    for core in range(NCORES):
        b, hg = divmod(core, 2)
        heads = list(range(hg * 8, hg * 8 + 8))
        in_maps.append(make_core_inputs(x[b], W_qkv, b_qkv, W_out, heads))

    res = run_bass_kernel_spmd(nc, in_maps, list(range(NCORES)), trace=_trace)
    kernel._last_results = res

    out = np.broadcast_to(b_out, (B, T, C)).astype(np.float32).copy()
    for core in range(NCORES):
        b = core // 2
        out[b] += res.results[core]["out"]
    return out
